# revision 1
# baseline (speedup 1.0000x reference)
"""Trainium2 Bass kernel: LISTA patch-denoiser with CBAM attention.

Sharding: 2 cores per image (4 images x 2 halves = 8 cores). Each core
owns a contiguous band of patch rows; core `2n+1` works on a vertically
flipped view of image `n` so both halves share one SPMD program (halo
rows at the end of the local frame, conv kernel row-flipped via input
data). Channel-attention pooling is made exact across the pair with one
tiny AllGather. The final overlap-add (fold) runs on the host as part of
the unshard step.

Device program per core:
  unfold(host) -> 4-layer MLP -> pooling stats -> AllGather(pair) ->
  channel attention -> spatial attention (channel sum via PE ones-matmul,
  channel max via GPSIMD partition_all_reduce, 7x7 conv as 14 banded
  matmuls) -> per-patch thresholds l -> 6 soft-thresholds (custom fused
  DVE op) interleaved with LISTA matmuls -> clipped reconstruction out.
"""
import sys
import os
import time

sys.path.insert(0, "/opt/trn_rl_repo")

import numpy as np


class _PhaseDone(Exception):
    pass


import concourse.bass as bass
import concourse.tile as tile
from concourse import bacc, mybir, bass_isa
from concourse.bass_utils import run_bass_kernel_spmd
from concourse.dve_spec import (Spec, Src0, Src1, C0, Zero, relu, maxx,
                                select, lower, _has_src1)
from concourse.dve_uop import DveOpSpec
import concourse.dve_ops as dve_ops

F32 = mybir.dt.float32
AF = mybir.ActivationFunctionType
ALU = mybir.AluOpType
AX = mybir.AxisListType

P = 8
T = 5
RE = 121            # patch grid side (128 - 8 + 1)
NROW = 64           # local patch rows per core (owned + halo)
NPAT = NROW * RE    # 7744
GS = 4 * RE         # 484 patches per group (4 patch rows)
NG = 16
HALF_G = 8          # ISTA runs in two 8-group passes to halve z SBUF
NCORES = 8
D, H1, H2, H3, DL = 64, 512, 256, 128, 256

_CACHE = {}
LAST_RESULTS = None
LAST_EXEC_WALL_S = None


# --------------------------------------------------------------------------
# custom fused DVE soft-threshold:  out = sign(v) * relu(|v| - l * (1/c))
# --------------------------------------------------------------------------
def _register_st_op():
    name = "ST_SOFTTHRESH_ANT"
    for o in dve_ops.OPS:
        if o.name == name:
            return o
    r = relu(maxx(Src0, Zero - Src0) - Src1 * C0)
    body = select(Src0 >= Zero, r, Zero - r)

    def _ref(in0, in1, s0, s1, imm2):
        rr = np.maximum(np.maximum(in0, -in0) - in1 * s0, 0.0)
        return np.where(in0 >= 0, rr, -rr).astype(np.float32)

    spec = Spec(body=body, reference=_ref)
    opcode = dve_ops._CUSTOM_DVE_ROW_BASE + len(dve_ops.OPS)
    shas = {}
    for ver in ("v3", "v4"):
        s = DveOpSpec(name=name, opcode=opcode, uops=lower(spec, ver=ver),
                      rd1_en=_has_src1(spec))
        shas[ver] = s.sha(ver)
    op = dve_ops.DveOp(name, spec, subdim=False, uops_sha=shas)
    dve_ops.OPS.append(op)
    dve_ops._SUB_OPCODE_FOR_NAME[name] = opcode
    dve_ops.CUSTOM_DVE_SPECS[name] = spec
    return op


# --------------------------------------------------------------------------
# device program
# --------------------------------------------------------------------------
def _build_nc(st_op):
    phases = int(os.environ.get("ST_PHASES", "9"))
    nc = bacc.Bacc("TRN2", target_bir_lowering=False, debug=False,
                   num_devices=NCORES)

    def din(name, shape, dt=F32):
        return nc.dram_tensor(name, shape, dt, kind="ExternalInput").ap()

    a_unf = din("unf", [D, NPAT])
    a_w1 = din("w1t", [D, H1])
    a_b1 = din("b1t", [128, 4])
    a_w2 = din("w2t", [128, 4 * H2])
    a_b2 = din("b2t", [128, 2])
    a_w3 = din("w3t", [128, 2 * H3])
    a_b3 = din("b3t", [128, 1])
    a_w4 = din("w4t", [128, DL])
    a_b4 = din("b4t", [128, 2])
    a_dct = din("dct", [D, DL])
    a_dcc = din("dcc", [D, DL])
    a_dtt = din("dtt", [128, 2 * D])
    a_s = din("st_", [128, 2 * DL])
    a_cw1s = din("cw1s", [128, 32])
    a_cw1 = din("cw1", [128, 32])
    a_cw2 = din("cw2", [16, DL])
    a_ones = din("ones1", [128, 1])
    a_invc = din("invc", [128, 1])
    a_band = din("band", [D, 14 * 64])
    a_mask = din("maskb", [128, NROW])
    a_imask = din("imask", [128, NROW], mybir.dt.int32)
    a_out = nc.dram_tensor("out", [128, NPAT], F32, kind="ExternalOutput").ap()

    with tile.TileContext(nc) as tc:
        import contextlib
        ctx = contextlib.ExitStack()
        try:
          with ctx:
            wp = ctx.enter_context(tc.tile_pool(name="wp", bufs=1))
            lamp = ctx.enter_context(tc.tile_pool(name="lamp", bufs=1))
            zp = ctx.enter_context(tc.tile_pool(name="zp", bufs=1))
            hp = ctx.enter_context(tc.tile_pool(name="hp", bufs=3))
            sp = ctx.enter_context(tc.tile_pool(name="sp", bufs=1))
            cb = ctx.enter_context(tc.tile_pool(name="cb", bufs=2))
            xpp = ctx.enter_context(tc.tile_pool(name="xpp", bufs=3))
            dp = ctx.enter_context(tc.tile_pool(name="dp", bufs=1,
                                                space="DRAM"))
            mlp_ctx = contextlib.ExitStack()
            mps1 = mlp_ctx.enter_context(tc.tile_pool(name="mps1", bufs=2,
                                                      space="PSUM"))
            mps2 = mlp_ctx.enter_context(tc.tile_pool(name="mps2", bufs=1,
                                                      space="PSUM"))
            mps34 = mlp_ctx.enter_context(tc.tile_pool(name="mps34", bufs=1,
                                                       space="PSUM"))

            # ---- load constants ----
            def wtile(shape, src, tag, dt=F32):
                t = wp.tile(shape, dt, tag=tag, name=tag)
                nc.sync.dma_start(t[:], src)
                return t

            w1 = wtile([D, H1], a_w1, "w1")
            b1 = wtile([128, 4], a_b1, "b1")
            unf_t = []
            for g in range(NG):
                u = wp.tile([D, GS], F32, tag=f"unf{g}", name=f"unf{g}")
                unf_t.append(u)
            for g in range(4):
                nc.sync.dma_start(unf_t[g][:],
                                  a_unf[:, g * GS:(g + 1) * GS])
            w2 = wtile([128, 4 * H2], a_w2, "w2")
            b2 = wtile([128, 2], a_b2, "b2")
            w3 = wtile([128, 2 * H3], a_w3, "w3")
            b3 = wtile([128, 1], a_b3, "b3")
            w4 = wtile([128, DL], a_w4, "w4")
            b4 = wtile([128, 2], a_b4, "b4")
            dct = wtile([D, DL], a_dct, "dct")
            dcc = wtile([D, DL], a_dcc, "dcc")
            dtt = wtile([128, 2 * D], a_dtt, "dtt")
            smat = wtile([128, 2 * DL], a_s, "smat")
            cw1s = wtile([128, 32], a_cw1s, "cw1s")
            cw1 = wtile([128, 32], a_cw1, "cw1")
            cw2 = wtile([16, DL], a_cw2, "cw2")
            ones1 = wtile([128, 1], a_ones, "ones1")
            invc = wtile([128, 1], a_invc, "invc")
            band = wtile([D, 14 * 64], a_band, "band")
            maskb = wtile([128, NROW], a_mask, "maskb")
            imask = wtile([128, NROW], a_imask, "imask", mybir.dt.int32)
            for g in range(4, NG):
                nc.sync.dma_start(unf_t[g][:],
                                  a_unf[:, g * GS:(g + 1) * GS])

            rowsum = [sp.tile([128, NROW], F32, tag=f"rsum{m}", name=f"rsum{m}")
                      for m in range(2)]
            rowmax = [sp.tile([128, NROW], F32, tag=f"rmax{m}", name=f"rmax{m}")
                      for m in range(2)]
            neginf = sp.tile([128, NROW], F32, tag="neginf", name="neginf")
            nc.gpsimd.memset(neginf[:], -3.0e38)

            lam_t = [[None] * NG, [None] * NG]

            # =========================== MLP ===========================
            for g in range(NG):
                gsl = slice(g * GS, (g + 1) * GS)
                ps2 = [mps2.tile([128, GS], F32, tag=f"ps2_{m}", name=f"ps2_{m}")
                       for m in range(2)]
                for kk in range(4):
                    ps1 = mps1.tile([128, GS], F32, tag="ps1", name="ps1")
                    nc.tensor.matmul(ps1[:], w1[:, kk * 128:(kk + 1) * 128],
                                     unf_t[g][:], start=True, stop=True)
                    h1k = hp.tile([128, GS], F32, tag="h1k", name="h1k")
                    if kk % 2 == 0:
                        nc.scalar.activation(h1k[:], ps1[:], AF.Relu,
                                             bias=b1[:, kk:kk + 1])
                    else:
                        nc.vector.tensor_scalar(h1k[:], ps1[:],
                                                b1[:, kk:kk + 1], 0.0,
                                                ALU.add, ALU.max)
                    for m in range(2):
                        o = kk * 2 * H3 + m * 128
                        nc.tensor.matmul(ps2[m][:], w2[:, o:o + 128],
                                         h1k[:], start=(kk == 0),
                                         stop=(kk == 3))
                h2t = []
                for m in range(2):
                    h2m = hp.tile([128, GS], F32, tag=f"h2_{m}", name=f"h2_{m}")
                    nc.scalar.activation(h2m[:], ps2[m][:], AF.Relu,
                                         bias=b2[:, m:m + 1])
                    h2t.append(h2m)
                ps3 = mps34.tile([128, GS], F32, tag="ps3", name="ps3", bufs=2)
                for kk in range(2):
                    nc.tensor.matmul(ps3[:], w3[:, kk * 128:(kk + 1) * 128],
                                     h2t[kk][:], start=(kk == 0),
                                     stop=(kk == 1))
                h3t = hp.tile([128, GS], F32, tag="h3", name="h3")
                nc.scalar.activation(h3t[:], ps3[:], AF.Relu, bias=b3[:, 0:1])
                for m in range(2):
                    ps4 = mps34.tile([128, GS], F32, tag=f"ps4_{m}", name=f"ps4_{m}")
                    nc.tensor.matmul(ps4[:], w4[:, m * 128:(m + 1) * 128],
                                     h3t[:], start=True, stop=True)
                    lam = lamp.tile([128, GS], F32, tag=f"lam{m}_{g}", name=f"lam{m}_{g}")
                    for r in range(4):
                        rsl = slice(r * RE, (r + 1) * RE)
                        nc.scalar.activation(
                            lam[:, rsl], ps4[:, rsl], AF.Identity,
                            bias=b4[:, m:m + 1],
                            accum_out=rowsum[m][:, g * 4 + r:g * 4 + r + 1])
                    lam_t[m][g] = lam
                    ap3 = lam[:].rearrange("p (r v) -> p r v", v=RE)
                    nc.vector.tensor_reduce(
                        rowmax[m][:, g * 4:(g + 1) * 4], ap3, axis=AX.X,
                        op=ALU.max)

            mlp_ctx.close()

            if phases <= 1:
                nc.sync.dma_start(a_out[:, 0:GS], lam_t[0][0][:])
                nc.sync.dma_start(a_out[:, GS:2 * GS], lam_t[1][0][:])
                nc.sync.dma_start(a_out[:, 2 * GS:2 * GS + NROW], rowsum[0][:])
                nc.sync.dma_start(a_out[:, 2 * GS + NROW:2 * GS + 2 * NROW],
                                  rowmax[0][:])
                raise _PhaseDone()

            bps_ctx = contextlib.ExitStack()
            bps = bps_ctx.enter_context(tc.tile_pool(name="bps", bufs=1,
                                                     space="PSUM"))

            # ================= pooling stats + AllGather ================
            mstat = sp.tile([128, 4], F32, tag="mstat", name="mstat")
            for m in range(2):
                t1 = sp.tile([128, NROW], F32, tag="scr1", name="scr1")
                nc.vector.tensor_tensor(t1[:], rowsum[m][:], maskb[:],
                                        op=ALU.mult)
                nc.vector.tensor_reduce(mstat[:, m:m + 1], t1[:], axis=AX.X,
                                        op=ALU.add)
                t2 = sp.tile([128, NROW], F32, tag="scr2", name="scr2")
                nc.vector.tensor_copy(t2[:], rowmax[m][:])
                nc.vector.copy_predicated(t2[:], imask[:], neginf[:])
                nc.vector.tensor_reduce(mstat[:, 2 + m:3 + m], t2[:],
                                        axis=AX.X, op=ALU.max)
            cc_in = dp.tile([128, 4], F32, name="cc_in")
            cc_out = dp.tile([1, 1024], F32, name="cc_out")
            nc.sync.dma_start(cc_in[:], mstat[:])
            nc.gpsimd.collective_compute(
                "AllGather", ALU.bypass,
                replica_groups=[[0, 1], [2, 3], [4, 5], [6, 7]],
                ins=[cc_in.opt()], outs=[cc_out.opt()])
            tg = sp.tile([128, 8], F32, tag="tg", name="tg")
            for hb in range(2):
                src = cc_out[0:1, hb * 512:(hb + 1) * 512].rearrange(
                    "a (p c) -> (a p) c", p=128, c=4)
                nc.sync.dma_start(tg[:, hb * 4:(hb + 1) * 4], src)
            st2 = sp.tile([128, 4], F32, tag="st2", name="st2")
            nc.vector.tensor_tensor(st2[:, 0:2], tg[:, 0:2], tg[:, 4:6],
                                    op=ALU.add)
            nc.vector.tensor_tensor(st2[:, 2:4], tg[:, 2:4], tg[:, 6:8],
                                    op=ALU.max)

            # ==================== channel attention =====================
            hbr = []
            for br, (wt, c0) in enumerate(((cw1s, 0), (cw1, 2))):
                psh = bps.tile([16, 1], F32, tag="psh", name="psh")
                for kk in range(2):
                    nc.tensor.matmul(psh[:], wt[:, kk * 16:(kk + 1) * 16],
                                     st2[:, c0 + kk:c0 + kk + 1],
                                     start=(kk == 0), stop=(kk == 1))
                hb_ = sp.tile([16, 1], F32, tag=f"hbr{br}", name=f"hbr{br}")
                nc.scalar.activation(hb_[:], psh[:], AF.Relu)
                hbr.append(hb_)
            ca = sp.tile([128, 2], F32, tag="ca", name="ca")
            for m in range(2):
                psca = bps.tile([128, 1], F32, tag="psca", name="psca")
                nc.tensor.matmul(psca[:], cw2[:, m * 128:(m + 1) * 128],
                                 hbr[0][:], start=True, stop=False)
                nc.tensor.matmul(psca[:], cw2[:, m * 128:(m + 1) * 128],
                                 hbr[1][:], start=False, stop=True)
                nc.scalar.activation(ca[:, m:m + 1], psca[:], AF.Sigmoid)

            if phases <= 2:
                nc.sync.dma_start(a_out[:, 0:4], mstat[:])
                nc.sync.dma_start(a_out[:, 4:12], tg[:])
                nc.sync.dma_start(a_out[:, 12:16], st2[:])
                nc.sync.dma_start(a_out[:, 16:18], ca[:])
                raise _PhaseDone()

            # ==================== spatial attention =====================
            mean_t = cb.tile([D, RE + 6], F32, tag="mean_t", name="mean_t")
            max_t = cb.tile([D, RE + 6], F32, tag="max_t", name="max_t")
            nc.gpsimd.memset(mean_t[:], 0.0)
            nc.gpsimd.memset(max_t[:], 0.0)
            for g in range(NG):
                for m in range(2):
                    lam = lam_t[m][g]
                    nc.scalar.activation(lam[:], lam[:], AF.Copy,
                                         scale=ca[:, m:m + 1])
                pss = bps.tile([1, GS], F32, tag="pss", name="pss")
                nc.tensor.matmul(pss[:], ones1[:], lam_t[0][g][:],
                                 start=True, stop=False)
                nc.tensor.matmul(pss[:], ones1[:], lam_t[1][g][:],
                                 start=False, stop=True)
                srs = cb.tile([1, GS], F32, tag="srs", name="srs")
                nc.scalar.activation(srs[:], pss[:], AF.Copy)
                nc.sync.dma_start(mean_t[4 * g:4 * g + 4, 3:3 + RE], srs[:])
                mx1 = cb.tile([128, GS], F32, tag="mx1", name="mx1")
                nc.vector.tensor_tensor(mx1[:], lam_t[0][g][:],
                                        lam_t[1][g][:], op=ALU.max)
                mx2 = cb.tile([128, GS], F32, tag="mx2", name="mx2")
                nc.gpsimd.partition_all_reduce(mx2[:], mx1[:], 128,
                                               bass_isa.ReduceOp.max)
                nc.sync.dma_start(max_t[4 * g:4 * g + 4, 3:3 + RE],
                                  mx2[0:1, :])
            psa = bps.tile([D, RE], F32, tag="psa", name="psa")
            idx = 0
            for dc, srct in enumerate((mean_t, max_t)):
                for dj in range(7):
                    o = (dc * 7 + dj) * 64
                    nc.tensor.matmul(psa[:], band[:, o:o + 64],
                                     srct[:, dj:dj + RE], start=(idx == 0),
                                     stop=(idx == 13))
                    idx += 1
            sa_sb = cb.tile([D, RE], F32, tag="sa_sb", name="sa_sb")
            nc.scalar.activation(sa_sb[:], psa[:], AF.Sigmoid)

            if phases <= 3:
                nc.sync.dma_start(a_out[0:D, 0:RE + 6], mean_t[:])
                nc.sync.dma_start(a_out[0:D, RE + 6:2 * (RE + 6)], max_t[:])
                nc.sync.dma_start(a_out[0:D, 2 * (RE + 6):2 * (RE + 6) + RE],
                                  sa_sb[:])
                raise _PhaseDone()

            bps_ctx.close()
            ipsv = ctx.enter_context(tc.tile_pool(name="ipsv", bufs=6,
                                                  space="PSUM"))
            ipsx = ctx.enter_context(tc.tile_pool(name="ipsx", bufs=2,
                                                  space="PSUM"))

            # ============ thresholds l (in lam tiles) + LISTA ===========
            z_t = [[None] * NG, [None] * NG]
            for half in range(2):
                gs_ = range(half * HALF_G, (half + 1) * HALF_G)
                for g in gs_:
                    srg = cb.tile([1, GS], F32, tag="srg", name="srg")
                    nc.sync.dma_start(srg[:], sa_sb[4 * g:4 * g + 4, 0:RE])
                    sab = cb.tile([128, GS], F32, tag="sab", name="sab")
                    nc.gpsimd.partition_broadcast(sab[:], srg[:], 128)
                    for m in range(2):
                        lam = lam_t[m][g]
                        nc.vector.tensor_tensor(lam[:], lam[:], sab[:],
                                                op=ALU.mult)
                if phases <= 4 and half == 0:
                    nc.sync.dma_start(a_out[:, 0:GS], lam_t[0][0][:])
                    nc.sync.dma_start(a_out[:, GS:2 * GS], lam_t[1][0][:])
                    raise _PhaseDone()
                # k = 0:  z = ST(unf @ Dict, l)
                for g in gs_:
                    w = RE if g == NG - 1 else GS
                    gsl = slice(g * GS, g * GS + w)
                    for m in range(2):
                        psv = ipsv.tile([128, GS], F32, tag="psv", name="psv")
                        nc.tensor.matmul(psv[:, 0:w],
                                         dct[:, m * 128:(m + 1) * 128],
                                         unf_t[g][:, 0:w], start=True,
                                         stop=True)
                        z = zp.tile([128, GS], F32, tag=f"z{m}_{g % HALF_G}", name=f"z{m}_{g % HALF_G}")
                        nc.vector._custom_dve(st_op, out=z[:, 0:w],
                                              in0=psv[:, 0:w],
                                              in1=lam_t[m][g][:, 0:w],
                                              s0=invc[:, 0:1])
                        z_t[m][g] = z
                # k = 1..T:  z = ST(z @ S + unf @ Dict/c, l)
                for k in range(T):
                    for g in gs_:
                        w = RE if g == NG - 1 else GS
                        gsl = slice(g * GS, g * GS + w)
                        psvs = []
                        for m in range(2):
                            psv = ipsv.tile([128, GS], F32, tag="psv", name="psv")
                            nc.tensor.matmul(
                                psv[:, 0:w], smat[:, m * 128:(m + 1) * 128],
                                z_t[0][g][:, 0:w], start=True, stop=False)
                            nc.tensor.matmul(
                                psv[:, 0:w],
                                smat[:, DL + m * 128:DL + (m + 1) * 128],
                                z_t[1][g][:, 0:w], start=False, stop=False)
                            nc.tensor.matmul(
                                psv[:, 0:w], dcc[:, m * 128:(m + 1) * 128],
                                unf_t[g][:, 0:w], start=False, stop=True)
                            psvs.append(psv)
                        for m in range(2):
                            nc.vector._custom_dve(st_op, out=z_t[m][g][:, 0:w],
                                                  in0=psvs[m][:, 0:w],
                                                  in1=lam_t[m][g][:, 0:w],
                                                  s0=invc[:, 0:1])
                # reconstruction
                for g in gs_:
                    w = RE if g == NG - 1 else GS
                    gsl = slice(g * GS, g * GS + w)
                    psx = ipsx.tile([D, GS], F32, tag="psx", name="psx")
                    nc.tensor.matmul(psx[:, 0:w], dtt[:, 0:D],
                                     z_t[0][g][:, 0:w],
                                     start=True, stop=False)
                    nc.tensor.matmul(psx[:, 0:w], dtt[:, D:2 * D],
                                     z_t[1][g][:, 0:w],
                                     start=False, stop=True)
                    xp = xpp.tile([D, GS], F32, tag="xp", name="xp")
                    nc.vector.tensor_scalar(xp[:, 0:w], psx[:, 0:w], 0.0, 1.0,
                                            ALU.max, ALU.min)
                    nc.sync.dma_start(a_out[0:D, gsl], xp[:, 0:w])

        except _PhaseDone:
            pass
    nc.compile()
    return nc


# --------------------------------------------------------------------------
# host-side data prep
# --------------------------------------------------------------------------
def _make_unf_T(img, half):
    """[64, NPAT] feature-major patches; components in image order.
    half 0: local row r = image patch row r; half 1: r -> 120 - r."""
    us = np.arange(NROW) if half == 0 else 120 - np.arange(NROW)
    i = np.arange(P)
    rows = us[:, None, None, None] + i[None, :, None, None]       # [64,8,1,1]
    cols = (np.arange(RE)[None, None, None, :]
            + np.arange(P)[None, None, :, None])                  # [1,1,8,121]
    pat = img[rows, cols]                                         # [64,8,8,121]
    return np.ascontiguousarray(
        pat.transpose(1, 2, 0, 3).reshape(D, NPAT)).astype(np.float32)


def _banded_conv(sa_conv, half):
    """14 banded [64,64] lhsT matrices: B[(dc,dj)][r,u] = W[dc, r-u+3, dj].
    Channel 0 (mean) carries the 1/256 mean normalization; half 1 uses the
    row-flipped kernel."""
    W = np.array(sa_conv[0], np.float32).copy()
    W[0] /= 256.0
    if half == 1:
        W = W[:, ::-1, :]
    out = np.zeros((D, 14 * 64), np.float32)
    r = np.arange(64)
    for dc in range(2):
        for dj in range(7):
            B = np.zeros((64, 64), np.float32)
            for u in range(64):
                di = r - u + 3
                ok = (di >= 0) & (di < 7)
                B[ok, u] = W[dc, di[ok], dj]
            out[:, (dc * 7 + dj) * 64:(dc * 7 + dj + 1) * 64] = B
    return out


def _host_inputs(inputs):
    x = np.asarray(inputs["x"], np.float32)
    Dict = np.asarray(inputs["Dict"], np.float32)
    cval = float(np.asarray(inputs["c"]))
    W1 = np.asarray(inputs["W1"], np.float32)
    W2 = np.asarray(inputs["W2"], np.float32)
    W3 = np.asarray(inputs["W3"], np.float32)
    W4 = np.asarray(inputs["W4"], np.float32)
    b1 = np.asarray(inputs["b1"], np.float32)
    b2 = np.asarray(inputs["b2"], np.float32)
    b3 = np.asarray(inputs["b3"], np.float32)
    b4 = np.asarray(inputs["b4"], np.float32)
    ca_w1 = np.asarray(inputs["ca_w1"], np.float32)
    ca_w2 = np.asarray(inputs["ca_w2"], np.float32)
    sa_conv = np.asarray(inputs["sa_conv"], np.float32)

    S = (np.eye(DL, dtype=np.float32) - (Dict.T @ Dict) / cval).T
    shared = dict(
        w1t=W1,
        b1t=np.ascontiguousarray(b1.reshape(4, 128).T),
        w2t=np.ascontiguousarray(np.hstack([W2[k * 128:(k + 1) * 128]
                                            for k in range(4)])),
        b2t=np.ascontiguousarray(b2.reshape(2, 128).T),
        w3t=np.ascontiguousarray(np.hstack([W3[k * 128:(k + 1) * 128]
                                            for k in range(2)])),
        b3t=np.ascontiguousarray(b3[:, None]),
        w4t=W4,
        b4t=np.ascontiguousarray(b4.reshape(2, 128).T),
        dct=Dict,
        dcc=np.ascontiguousarray(Dict / cval),
        dtt=np.ascontiguousarray(np.hstack([Dict.T[k * 128:(k + 1) * 128]
                                            for k in range(2)])),
        st_=np.ascontiguousarray(np.hstack([S[k * 128:(k + 1) * 128]
                                            for k in range(2)])),
        cw1s=np.ascontiguousarray(np.hstack(
            [(ca_w1 / float(RE * RE))[k * 128:(k + 1) * 128]
             for k in range(2)])),
        cw1=np.ascontiguousarray(np.hstack([ca_w1[k * 128:(k + 1) * 128]
                                            for k in range(2)])),
        cw2=ca_w2,
        ones1=np.ones((128, 1), np.float32),
        invc=np.full((128, 1), 1.0 / cval, np.float32),
    )
    in_maps = []
    for c in range(NCORES):
        n, half = c // 2, c % 2
        nown = 61 if half == 0 else 60
        mk = np.zeros((128, NROW), np.float32)
        mk[:, :nown] = 1.0
        m = dict(shared)
        m["unf"] = _make_unf_T(x[n, 0], half)
        m["band"] = _banded_conv(sa_conv, half)
        m["maskb"] = mk
        m["imask"] = np.ascontiguousarray((1.0 - mk).astype(np.int32))
        in_maps.append(m)
    return in_maps


_COUNT = None


def _fold_count():
    global _COUNT
    if _COUNT is None:
        cnt = np.zeros((128, 128), np.float32)
        for i in range(P):
            for j in range(P):
                cnt[i:i + RE, j:j + RE] += 1.0
        _COUNT = cnt
    return _COUNT


def _host_fold(xps, wval):
    count = _fold_count()
    out = np.zeros((4, 1, 128, 128), np.float32)
    for n in range(4):
        acc = np.zeros((128, 128), np.float32)
        for half in (0, 1):
            xp = xps[2 * n + half].reshape(D, NROW, RE)
            nrows = 61 if half == 0 else 60
            pl = xp[:, :nrows, :]
            for i in range(P):
                for j in range(P):
                    plane = pl[i * P + j]
                    if half == 0:
                        acc[i:i + nrows, j:j + RE] += plane
                    else:
                        acc[61 + i:121 + i, j:j + RE] += plane[::-1, :]
        out[n, 0] = (acc * wval) / (wval * count)
    return out


def kernel(**inputs) -> np.ndarray:
    global LAST_RESULTS
    st_op = _register_st_op()
    if "nc" not in _CACHE:
        _CACHE["nc"] = _build_nc(st_op)
    nc = _CACHE["nc"]
    in_maps = _host_inputs(inputs)
    global LAST_EXEC_WALL_S
    t0 = time.time()
    res = run_bass_kernel_spmd(nc, in_maps, core_ids=list(range(NCORES)))
    LAST_EXEC_WALL_S = time.time() - t0
    LAST_RESULTS = res
    wval = float(np.asarray(inputs["w"]))
    xps = [res.results[c]["out"][:D] for c in range(NCORES)]
    return _host_fold(xps, wval)



# revision 2
# speedup vs baseline: 6.2595x; 6.2595x over previous
"""Trainium2 Bass kernel: LISTA patch-denoiser with CBAM attention.

Sharding: 2 cores per image (4 images x 2 halves = 8 cores). Each core
owns a contiguous band of patch rows; core `2n+1` works on a vertically
flipped view of image `n` so both halves share one SPMD program (all
per-half differences — row flip, feature-order i-reversal, conv-kernel
row flip, row masks — are absorbed into the per-core input data).

Transfer-minimized design: each core receives ONE packed f32 tensor
(~1.7 MB: weights + raw 71x128 half-image). Unfold runs on-device via
overlapping strided DMAs; the final overlap-add (fold) runs on-device
via shifted-lhsT PSUM-accumulating matmuls, so the per-core output is a
[68,128] partial image instead of [128,7744] patches. The host only
stitches the two half-images and divides by the coverage count.

Device program per core:
  unfold (8 strided DMAs) -> 4-layer MLP -> pooling stats ->
  AllGather(pair) -> channel attention -> spatial attention (7x7 conv as
  14 banded matmuls) -> per-patch thresholds l -> 6 soft-thresholds
  (custom fused DVE op) interleaved with LISTA matmuls -> clipped
  reconstruction -> on-device fold -> [68,128] partial image out.
"""
import sys
import os
import time

sys.path.insert(0, "/opt/trn_rl_repo")

import numpy as np


class _PhaseDone(Exception):
    pass


import concourse.bass as bass
import concourse.tile as tile
from concourse import bacc, mybir, bass_isa
from concourse.bass_utils import run_bass_kernel_spmd
from concourse.dve_spec import (Spec, Src0, Src1, C0, Zero, relu, maxx,
                                select, lower, _has_src1)
from concourse.dve_uop import DveOpSpec
import concourse.dve_ops as dve_ops
import bass_rust

F32 = mybir.dt.float32
AF = mybir.ActivationFunctionType
ALU = mybir.AluOpType
AX = mybir.AxisListType
VP = bass_rust.VecI64Pair

P = 8
T = 5
RE = 121            # patch grid side (128 - 8 + 1)
NROW = 64           # local patch rows per core (owned + halo)
NPAT = NROW * RE    # 7744
GS = 4 * RE         # 484 patches per group (4 patch rows)
NG = 16
HALF_G = 8          # ISTA runs in two 8-group passes to halve z SBUF
NCORES = 8
D, H1, H2, H3, DL = 64, 512, 256, 128, 256
IMG_ROWS = 71       # local image rows needed: 64 patch rows + 7

# packed per-core input layout: (name, partitions, cols)
_LAYOUT = [
    ("w1t", 64, 512), ("b1t", 128, 4), ("w2t", 128, 1024), ("b2t", 128, 2),
    ("w3t", 128, 256), ("b3t", 128, 1), ("w4t", 128, 256), ("b4t", 128, 2),
    ("dct", 64, 256), ("dcc", 64, 256), ("dtt", 128, 128), ("st_", 128, 512),
    ("cw1s", 128, 32), ("cw1", 128, 32), ("cw2", 16, 256),
    ("ones1", 128, 1), ("invc", 128, 1), ("band", 64, 896),
    ("maskb", 128, 64), ("sel8", 64, 8), ("rowm61", 128, 1),
    ("img", IMG_ROWS, 128),
]
_OFFS = {}
_NTOT = 0
for _n, _p, _c in _LAYOUT:
    _OFFS[_n] = _NTOT
    _NTOT += _p * _c

_CACHE = {}
LAST_RESULTS = None
LAST_EXEC_WALL_S = None


# --------------------------------------------------------------------------
# custom fused DVE soft-threshold:  out = sign(v) * relu(|v| - l * (1/c))
# --------------------------------------------------------------------------
def _register_st_op():
    name = "ST_SOFTTHRESH_ANT"
    for o in dve_ops.OPS:
        if o.name == name:
            return o
    r = relu(maxx(Src0, Zero - Src0) - Src1 * C0)
    body = select(Src0 >= Zero, r, Zero - r)

    def _ref(in0, in1, s0, s1, imm2):
        rr = np.maximum(np.maximum(in0, -in0) - in1 * s0, 0.0)
        return np.where(in0 >= 0, rr, -rr).astype(np.float32)

    spec = Spec(body=body, reference=_ref)
    opcode = dve_ops._CUSTOM_DVE_ROW_BASE + len(dve_ops.OPS)
    shas = {}
    for ver in ("v3", "v4"):
        s = DveOpSpec(name=name, opcode=opcode, uops=lower(spec, ver=ver),
                      rd1_en=_has_src1(spec))
        shas[ver] = s.sha(ver)
    op = dve_ops.DveOp(name, spec, subdim=False, uops_sha=shas)
    dve_ops.OPS.append(op)
    dve_ops._SUB_OPCODE_FOR_NAME[name] = opcode
    dve_ops.CUSTOM_DVE_SPECS[name] = spec
    return op


# --------------------------------------------------------------------------
# device program
# --------------------------------------------------------------------------
def _build_nc(st_op):
    phases = int(os.environ.get("ST_PHASES", "9"))
    nc = bacc.Bacc("TRN2", target_bir_lowering=False, debug=False,
                   num_devices=NCORES)

    a_pk = nc.dram_tensor("pk", [1, _NTOT], F32, kind="ExternalInput").ap()
    a_out = nc.dram_tensor("out", [68, 128], F32, kind="ExternalOutput").ap()
    a_dbg = None
    if phases < 9:
        a_dbg = nc.dram_tensor("dbg", [128, 1024], F32,
                               kind="ExternalOutput").ap()

    with tile.TileContext(nc) as tc:
        import contextlib
        ctx = contextlib.ExitStack()
        try:
          with ctx:
            wp = ctx.enter_context(tc.tile_pool(name="wp", bufs=1))
            lamp = ctx.enter_context(tc.tile_pool(name="lamp", bufs=1))
            zp = ctx.enter_context(tc.tile_pool(name="zp", bufs=1))
            hp = ctx.enter_context(tc.tile_pool(name="hp", bufs=3))
            sp = ctx.enter_context(tc.tile_pool(name="sp", bufs=1))
            cb = ctx.enter_context(tc.tile_pool(name="cb", bufs=2))
            xpp = ctx.enter_context(tc.tile_pool(name="xpp", bufs=3))
            fip = ctx.enter_context(tc.tile_pool(name="fip", bufs=2))
            dp = ctx.enter_context(tc.tile_pool(name="dp", bufs=1,
                                                space="DRAM"))
            mlp_ctx = contextlib.ExitStack()
            mps1 = mlp_ctx.enter_context(tc.tile_pool(name="mps1", bufs=2,
                                                      space="PSUM"))
            mps2 = mlp_ctx.enter_context(tc.tile_pool(name="mps2", bufs=1,
                                                      space="PSUM"))
            mps34 = mlp_ctx.enter_context(tc.tile_pool(name="mps34", bufs=1,
                                                       space="PSUM"))

            # ---- load constants from the packed input ----
            def wtile(name, dt=F32):
                _, p_, c_ = next(e for e in _LAYOUT if e[0] == name)
                t = wp.tile([p_, c_], dt, tag=name, name=name)
                off = _OFFS[name]
                nc.sync.dma_start(
                    t[:], a_pk[0:1, off:off + p_ * c_].rearrange(
                        "a (p c) -> (a p) c", p=p_, c=c_))
                return t

            w1 = wtile("w1t")
            b1 = wtile("b1t")

            # ---- on-device unfold: ufull[(j*8+i), r*121+v] = img[r+i, j+v]
            ufull = wp.tile([D, NPAT], F32, tag="ufull", name="ufull")
            for j in range(P):
                src = a_pk.copy()
                src.offset = _OFFS["img"] + j
                src.ap = VP([[128, P], [128, NROW], [1, RE]])  # i, r, v
                nc.sync.dma_start(ufull[j * P:(j + 1) * P, :], src)

            def unf_v(g):
                return ufull[:, g * GS:(g + 1) * GS]

            w2 = wtile("w2t")
            b2 = wtile("b2t")
            w3 = wtile("w3t")
            b3 = wtile("b3t")
            w4 = wtile("w4t")
            b4 = wtile("b4t")
            dct = wtile("dct")
            dcc = wtile("dcc")
            dtt = wtile("dtt")
            smat = wtile("st_")
            cw1s = wtile("cw1s")
            cw1 = wtile("cw1")
            cw2 = wtile("cw2")
            ones1 = wtile("ones1")
            invc = wtile("invc")
            band = wtile("band")
            maskb = wtile("maskb")
            sel8 = wtile("sel8")
            rowm61 = wtile("rowm61")

            # selw [64,128]: zeros except selw[j*8+i, 60+i] = 1
            selw = wp.tile([D, 128], F32, tag="selw", name="selw")
            nc.gpsimd.memset(selw[:], 0.0)
            nc.sync.dma_start(selw[:, 60:68], sel8[:])

            rowsum = [sp.tile([128, NROW], F32, tag=f"rsum{m}", name=f"rsum{m}")
                      for m in range(2)]
            rowmax = [sp.tile([128, NROW], F32, tag=f"rmax{m}", name=f"rmax{m}")
                      for m in range(2)]

            lam_t = [[None] * NG, [None] * NG]

            if phases <= 0 and a_dbg is not None:
                nc.sync.dma_start(a_dbg[0:D, 0:GS], unf_v(0))
                nc.sync.dma_start(a_dbg[0:D, GS:2 * GS], unf_v(15))
                raise _PhaseDone()

            # =========================== MLP ===========================
            for g in range(NG):
                ps2 = [mps2.tile([128, GS], F32, tag=f"ps2_{m}", name=f"ps2_{m}")
                       for m in range(2)]
                for kk in range(4):
                    ps1 = mps1.tile([128, GS], F32, tag="ps1", name="ps1")
                    nc.tensor.matmul(ps1[:], w1[:, kk * 128:(kk + 1) * 128],
                                     unf_v(g), start=True, stop=True)
                    h1k = hp.tile([128, GS], F32, tag="h1k", name="h1k")
                    if kk % 2 == 0:
                        nc.scalar.activation(h1k[:], ps1[:], AF.Relu,
                                             bias=b1[:, kk:kk + 1])
                    else:
                        nc.vector.tensor_scalar(h1k[:], ps1[:],
                                                b1[:, kk:kk + 1], 0.0,
                                                ALU.add, ALU.max)
                    for m in range(2):
                        o = kk * 2 * H3 + m * 128
                        nc.tensor.matmul(ps2[m][:], w2[:, o:o + 128],
                                         h1k[:], start=(kk == 0),
                                         stop=(kk == 3))
                h2t = []
                for m in range(2):
                    h2m = hp.tile([128, GS], F32, tag=f"h2_{m}", name=f"h2_{m}")
                    nc.scalar.activation(h2m[:], ps2[m][:], AF.Relu,
                                         bias=b2[:, m:m + 1])
                    h2t.append(h2m)
                ps3 = mps34.tile([128, GS], F32, tag="ps3", name="ps3", bufs=2)
                for kk in range(2):
                    nc.tensor.matmul(ps3[:], w3[:, kk * 128:(kk + 1) * 128],
                                     h2t[kk][:], start=(kk == 0),
                                     stop=(kk == 1))
                h3t = hp.tile([128, GS], F32, tag="h3", name="h3")
                nc.scalar.activation(h3t[:], ps3[:], AF.Relu, bias=b3[:, 0:1])
                for m in range(2):
                    ps4 = mps34.tile([128, GS], F32, tag=f"ps4_{m}", name=f"ps4_{m}")
                    nc.tensor.matmul(ps4[:], w4[:, m * 128:(m + 1) * 128],
                                     h3t[:], start=True, stop=True)
                    lam = lamp.tile([128, GS], F32, tag=f"lam{m}_{g}", name=f"lam{m}_{g}")
                    for r in range(4):
                        rsl = slice(r * RE, (r + 1) * RE)
                        nc.scalar.activation(
                            lam[:, rsl], ps4[:, rsl], AF.Identity,
                            bias=b4[:, m:m + 1],
                            accum_out=rowsum[m][:, g * 4 + r:g * 4 + r + 1])
                    lam_t[m][g] = lam
                    ap3 = lam[:].rearrange("p (r v) -> p r v", v=RE)
                    nc.vector.tensor_reduce(
                        rowmax[m][:, g * 4:(g + 1) * 4], ap3, axis=AX.X,
                        op=ALU.max)

            mlp_ctx.close()

            if phases <= 1 and a_dbg is not None:
                nc.sync.dma_start(a_dbg[:, 0:GS], lam_t[0][0][:])
                nc.sync.dma_start(a_dbg[:, GS:2 * GS], lam_t[1][0][:])
                nc.sync.dma_start(a_dbg[:, 2 * GS:2 * GS + NROW], rowsum[0][:])
                nc.sync.dma_start(a_dbg[:, 2 * GS + NROW:2 * GS + 2 * NROW],
                                  rowmax[0][:])
                raise _PhaseDone()

            bps_ctx = contextlib.ExitStack()
            bps = bps_ctx.enter_context(tc.tile_pool(name="bps", bufs=1,
                                                     space="PSUM"))

            # ================= pooling stats + AllGather ================
            # mneg: 0 where row owned, -1e38 where not
            mneg = sp.tile([128, NROW], F32, tag="mneg", name="mneg")
            nc.vector.tensor_scalar(mneg[:], maskb[:], -1.0, 1.0e38,
                                    ALU.add, ALU.mult)
            mstat = sp.tile([128, 4], F32, tag="mstat", name="mstat")
            for m in range(2):
                t1 = sp.tile([128, NROW], F32, tag="scr1", name="scr1")
                nc.vector.tensor_tensor(t1[:], rowsum[m][:], maskb[:],
                                        op=ALU.mult)
                nc.vector.tensor_reduce(mstat[:, m:m + 1], t1[:], axis=AX.X,
                                        op=ALU.add)
                t2 = sp.tile([128, NROW], F32, tag="scr2", name="scr2")
                nc.vector.tensor_tensor(t2[:], rowmax[m][:], mneg[:],
                                        op=ALU.add)
                nc.vector.tensor_reduce(mstat[:, 2 + m:3 + m], t2[:],
                                        axis=AX.X, op=ALU.max)
            cc_in = dp.tile([128, 4], F32, name="cc_in")
            cc_out = dp.tile([1, 1024], F32, name="cc_out")
            nc.sync.dma_start(cc_in[:], mstat[:])
            nc.gpsimd.collective_compute(
                "AllGather", ALU.bypass,
                replica_groups=[[0, 1], [2, 3], [4, 5], [6, 7]],
                ins=[cc_in.opt()], outs=[cc_out.opt()])
            tg = sp.tile([128, 8], F32, tag="tg", name="tg")
            for hb in range(2):
                src = cc_out[0:1, hb * 512:(hb + 1) * 512].rearrange(
                    "a (p c) -> (a p) c", p=128, c=4)
                nc.sync.dma_start(tg[:, hb * 4:(hb + 1) * 4], src)
            st2 = sp.tile([128, 4], F32, tag="st2", name="st2")
            nc.vector.tensor_tensor(st2[:, 0:2], tg[:, 0:2], tg[:, 4:6],
                                    op=ALU.add)
            nc.vector.tensor_tensor(st2[:, 2:4], tg[:, 2:4], tg[:, 6:8],
                                    op=ALU.max)

            # ==================== channel attention =====================
            hbr = []
            for br, (wt, c0) in enumerate(((cw1s, 0), (cw1, 2))):
                psh = bps.tile([16, 1], F32, tag="psh", name="psh")
                for kk in range(2):
                    nc.tensor.matmul(psh[:], wt[:, kk * 16:(kk + 1) * 16],
                                     st2[:, c0 + kk:c0 + kk + 1],
                                     start=(kk == 0), stop=(kk == 1))
                hb_ = sp.tile([16, 1], F32, tag=f"hbr{br}", name=f"hbr{br}")
                nc.scalar.activation(hb_[:], psh[:], AF.Relu)
                hbr.append(hb_)
            ca = sp.tile([128, 2], F32, tag="ca", name="ca")
            for m in range(2):
                psca = bps.tile([128, 1], F32, tag="psca", name="psca")
                nc.tensor.matmul(psca[:], cw2[:, m * 128:(m + 1) * 128],
                                 hbr[0][:], start=True, stop=False)
                nc.tensor.matmul(psca[:], cw2[:, m * 128:(m + 1) * 128],
                                 hbr[1][:], start=False, stop=True)
                nc.scalar.activation(ca[:, m:m + 1], psca[:], AF.Sigmoid)

            if phases <= 2 and a_dbg is not None:
                nc.sync.dma_start(a_dbg[:, 0:4], mstat[:])
                nc.sync.dma_start(a_dbg[:, 4:12], tg[:])
                nc.sync.dma_start(a_dbg[:, 12:16], st2[:])
                nc.sync.dma_start(a_dbg[:, 16:18], ca[:])
                raise _PhaseDone()

            # ==================== spatial attention =====================
            mean_t = cb.tile([D, RE + 6], F32, tag="mean_t", name="mean_t")
            max_t = cb.tile([D, RE + 6], F32, tag="max_t", name="max_t")
            nc.gpsimd.memset(mean_t[:], 0.0)
            nc.gpsimd.memset(max_t[:], 0.0)
            for g in range(NG):
                for m in range(2):
                    lam = lam_t[m][g]
                    nc.scalar.activation(lam[:], lam[:], AF.Copy,
                                         scale=ca[:, m:m + 1])
                pss = bps.tile([1, GS], F32, tag="pss", name="pss")
                nc.tensor.matmul(pss[:], ones1[:], lam_t[0][g][:],
                                 start=True, stop=False)
                nc.tensor.matmul(pss[:], ones1[:], lam_t[1][g][:],
                                 start=False, stop=True)
                srs = cb.tile([1, GS], F32, tag="srs", name="srs")
                nc.scalar.activation(srs[:], pss[:], AF.Copy)
                nc.sync.dma_start(mean_t[4 * g:4 * g + 4, 3:3 + RE], srs[:])
                mx1 = cb.tile([128, GS], F32, tag="mx1", name="mx1")
                nc.vector.tensor_tensor(mx1[:], lam_t[0][g][:],
                                        lam_t[1][g][:], op=ALU.max)
                mx2 = cb.tile([128, GS], F32, tag="mx2", name="mx2")
                nc.gpsimd.partition_all_reduce(mx2[:], mx1[:], 128,
                                               bass_isa.ReduceOp.max)
                nc.sync.dma_start(max_t[4 * g:4 * g + 4, 3:3 + RE],
                                  mx2[0:1, :])
            psa = bps.tile([D, RE], F32, tag="psa", name="psa")
            idx = 0
            for dc, srct in enumerate((mean_t, max_t)):
                for dj in range(7):
                    o = (dc * 7 + dj) * 64
                    nc.tensor.matmul(psa[:], band[:, o:o + 64],
                                     srct[:, dj:dj + RE], start=(idx == 0),
                                     stop=(idx == 13))
                    idx += 1
            sa_sb = cb.tile([D, RE], F32, tag="sa_sb", name="sa_sb")
            nc.scalar.activation(sa_sb[:], psa[:], AF.Sigmoid)

            if phases <= 3 and a_dbg is not None:
                nc.sync.dma_start(a_dbg[0:D, 0:RE + 6], mean_t[:])
                nc.sync.dma_start(a_dbg[0:D, RE + 6:2 * (RE + 6)], max_t[:])
                nc.sync.dma_start(a_dbg[0:D, 2 * (RE + 6):2 * (RE + 6) + RE],
                                  sa_sb[:])
                raise _PhaseDone()

            bps_ctx.close()
            ipsv = ctx.enter_context(tc.tile_pool(name="ipsv", bufs=5,
                                                  space="PSUM"))
            ipsx = ctx.enter_context(tc.tile_pool(name="ipsx", bufs=2,
                                                  space="PSUM"))
            fps = ctx.enter_context(tc.tile_pool(name="fps", bufs=1,
                                                 space="PSUM"))
            pf = fps.tile([68, 128], F32, tag="pf", name="pf")

            # ============ thresholds l (in lam tiles) + LISTA ===========
            z_t = [[None] * NG, [None] * NG]
            for half in range(2):
                gs_ = range(half * HALF_G, (half + 1) * HALF_G)
                for g in gs_:
                    srg = cb.tile([1, GS], F32, tag="srg", name="srg")
                    nc.sync.dma_start(srg[:], sa_sb[4 * g:4 * g + 4, 0:RE])
                    sab = cb.tile([128, GS], F32, tag="sab", name="sab")
                    nc.gpsimd.partition_broadcast(sab[:], srg[:], 128)
                    for m in range(2):
                        lam = lam_t[m][g]
                        nc.vector.tensor_tensor(lam[:], lam[:], sab[:],
                                                op=ALU.mult)
                if phases <= 4 and half == 0 and a_dbg is not None:
                    nc.sync.dma_start(a_dbg[:, 0:GS], lam_t[0][0][:])
                    nc.sync.dma_start(a_dbg[:, GS:2 * GS], lam_t[1][0][:])
                    raise _PhaseDone()
                # k = 0:  z = ST(unf @ Dict, l)
                for g in gs_:
                    w = RE if g == NG - 1 else GS
                    for m in range(2):
                        psv = ipsv.tile([128, GS], F32, tag="psv", name="psv")
                        nc.tensor.matmul(psv[:, 0:w],
                                         dct[:, m * 128:(m + 1) * 128],
                                         unf_v(g)[:, 0:w], start=True,
                                         stop=True)
                        z = zp.tile([128, GS], F32, tag=f"z{m}_{g % HALF_G}", name=f"z{m}_{g % HALF_G}")
                        nc.vector._custom_dve(st_op, out=z[:, 0:w],
                                              in0=psv[:, 0:w],
                                              in1=lam_t[m][g][:, 0:w],
                                              s0=invc[:, 0:1])
                        z_t[m][g] = z
                # k = 1..T:  z = ST(z @ S + unf @ Dict/c, l)
                for k in range(T):
                    for g in gs_:
                        w = RE if g == NG - 1 else GS
                        psvs = []
                        for m in range(2):
                            psv = ipsv.tile([128, GS], F32, tag="psv", name="psv")
                            nc.tensor.matmul(
                                psv[:, 0:w], smat[:, m * 128:(m + 1) * 128],
                                z_t[0][g][:, 0:w], start=True, stop=False)
                            nc.tensor.matmul(
                                psv[:, 0:w],
                                smat[:, DL + m * 128:DL + (m + 1) * 128],
                                z_t[1][g][:, 0:w], start=False, stop=False)
                            nc.tensor.matmul(
                                psv[:, 0:w], dcc[:, m * 128:(m + 1) * 128],
                                unf_v(g)[:, 0:w], start=False, stop=True)
                            psvs.append(psv)
                        for m in range(2):
                            nc.vector._custom_dve(st_op, out=z_t[m][g][:, 0:w],
                                                  in0=psvs[m][:, 0:w],
                                                  in1=lam_t[m][g][:, 0:w],
                                                  s0=invc[:, 0:1])
                # reconstruction + on-device fold
                for g in gs_:
                    w = RE if g == NG - 1 else GS
                    nr = 1 if g == NG - 1 else 4
                    psx = ipsx.tile([D, GS], F32, tag="psx", name="psx")
                    nc.tensor.matmul(psx[:, 0:w], dtt[:, 0:D],
                                     z_t[0][g][:, 0:w],
                                     start=True, stop=False)
                    nc.tensor.matmul(psx[:, 0:w], dtt[:, D:2 * D],
                                     z_t[1][g][:, 0:w],
                                     start=False, stop=True)
                    xp = xpp.tile([D, GS], F32, tag="xp", name="xp")
                    nc.vector.tensor_scalar(xp[:, 0:w], psx[:, 0:w], 0.0, 1.0,
                                            ALU.max, ALU.min)
                    if g == NG - 1:
                        # mask patch row 60 on half-1 cores (owned by half 0)
                        nc.scalar.activation(xp[:, 0:w], xp[:, 0:w], AF.Copy,
                                             scale=rowm61[0:D, 0:1])
                    fi = fip.tile([D, 4 * 128], F32, tag="fi", name="fi")
                    nc.gpsimd.memset(fi[:, 0:nr * 128], 0.0)
                    fi3 = fi[:].rearrange("p (r c) -> p r c", c=128)
                    xp3 = xp[:, 0:nr * RE].rearrange("p (r v) -> p r v", v=RE)
                    for j in range(P):
                        nc.sync.dma_start(
                            fi3[j * P:(j + 1) * P, 0:nr, j:j + RE],
                            xp3[j * P:(j + 1) * P, :, :])
                    for r in range(nr):
                        gr = g * 4 + r
                        nc.tensor.matmul(pf[:, :],
                                         selw[:, 60 - gr:128 - gr],
                                         fi[:, r * 128:(r + 1) * 128],
                                         start=(gr == 0), stop=(gr == 60))

            outt = xpp.tile([68, 128], F32, tag="outt", name="outt")
            nc.scalar.activation(outt[:], pf[:], AF.Copy)
            nc.sync.dma_start(a_out[:, :], outt[:])

        except _PhaseDone:
            pass
    nc.compile()
    return nc


# --------------------------------------------------------------------------
# host-side data prep
# --------------------------------------------------------------------------
def _banded_conv(sa_conv, half):
    """14 banded [64,64] lhsT matrices: B[(dc,dj)][r,u] = W[dc, r-u+3, dj].
    Channel 0 (mean) carries the 1/256 mean normalization; half 1 uses the
    row-flipped kernel."""
    W = np.array(sa_conv[0], np.float32).copy()
    W[0] /= 256.0
    if half == 1:
        W = W[:, ::-1, :]
    out = np.zeros((D, 14 * 64), np.float32)
    r = np.arange(64)
    for dc in range(2):
        for dj in range(7):
            B = np.zeros((64, 64), np.float32)
            for u in range(64):
                di = r - u + 3
                ok = (di >= 0) & (di < 7)
                B[ok, u] = W[dc, di[ok], dj]
            out[:, (dc * 7 + dj) * 64:(dc * 7 + dj + 1) * 64] = B
    return out


def _half_pack(inputs, half):
    """Weight section of the packed input for one half (h0 or h1)."""
    Dict = np.asarray(inputs["Dict"], np.float32)
    cval = float(np.asarray(inputs["c"]))
    W1 = np.asarray(inputs["W1"], np.float32)
    W2 = np.asarray(inputs["W2"], np.float32)
    W3 = np.asarray(inputs["W3"], np.float32)
    W4 = np.asarray(inputs["W4"], np.float32)
    b1 = np.asarray(inputs["b1"], np.float32)
    b2 = np.asarray(inputs["b2"], np.float32)
    b3 = np.asarray(inputs["b3"], np.float32)
    b4 = np.asarray(inputs["b4"], np.float32)
    ca_w1 = np.asarray(inputs["ca_w1"], np.float32)
    ca_w2 = np.asarray(inputs["ca_w2"], np.float32)
    sa_conv = np.asarray(inputs["sa_conv"], np.float32)

    # feature order f' = j*8+i; half 1 reverses i (flipped image rows)
    perm = np.array([(i if half == 0 else 7 - i) * P + j
                     for j in range(P) for i in range(P)])
    S = (np.eye(DL, dtype=np.float32) - (Dict.T @ Dict) / cval).T
    DT = Dict.T[:, perm]
    sel8 = np.zeros((D, 8), np.float32)
    for j in range(P):
        for i in range(P):
            sel8[j * P + i, i] = 1.0
    nown = 61 - half
    mk = np.zeros((128, NROW), np.float32)
    mk[:, :nown] = 1.0

    vals = dict(
        w1t=W1[perm],
        b1t=b1.reshape(4, 128).T,
        w2t=np.hstack([W2[k * 128:(k + 1) * 128] for k in range(4)]),
        b2t=b2.reshape(2, 128).T,
        w3t=np.hstack([W3[k * 128:(k + 1) * 128] for k in range(2)]),
        b3t=b3[:, None],
        w4t=W4,
        b4t=b4.reshape(2, 128).T,
        dct=Dict[perm],
        dcc=Dict[perm] / cval,
        dtt=np.hstack([DT[k * 128:(k + 1) * 128] for k in range(2)]),
        st_=np.hstack([S[k * 128:(k + 1) * 128] for k in range(2)]),
        cw1s=np.hstack([(ca_w1 / float(RE * RE))[k * 128:(k + 1) * 128]
                        for k in range(2)]),
        cw1=np.hstack([ca_w1[k * 128:(k + 1) * 128] for k in range(2)]),
        cw2=ca_w2,
        ones1=np.ones((128, 1), np.float32),
        invc=np.full((128, 1), 1.0 / cval, np.float32),
        band=_banded_conv(sa_conv, half),
        maskb=mk,
        sel8=sel8,
        rowm61=np.full((128, 1), 1.0 - half, np.float32),
    )
    parts = []
    for name, p_, c_ in _LAYOUT:
        if name == "img":
            continue
        v = np.ascontiguousarray(vals[name], np.float32)
        assert v.shape == (p_, c_), (name, v.shape, (p_, c_))
        parts.append(v.ravel())
    return np.concatenate(parts)


def _host_inputs(inputs):
    x = np.asarray(inputs["x"], np.float32)
    wsec = [_half_pack(inputs, h) for h in range(2)]
    in_maps = []
    for c in range(NCORES):
        n, half = c // 2, c % 2
        if half == 0:
            img = x[n, 0, 0:IMG_ROWS, :]
        else:
            img = x[n, 0, 128 - IMG_ROWS:128, :][::-1]
        pk = np.concatenate([wsec[half], np.ascontiguousarray(img).ravel()])
        in_maps.append({"pk": pk.reshape(1, -1)})
    return in_maps


_COUNT = None


def _fold_count():
    global _COUNT
    if _COUNT is None:
        cnt = np.zeros((128, 128), np.float32)
        for i in range(P):
            for j in range(P):
                cnt[i:i + RE, j:j + RE] += 1.0
        _COUNT = cnt
    return _COUNT


def _host_stitch(outs):
    count = _fold_count()
    res = np.empty((4, 1, 128, 128), np.float32)
    for n in range(4):
        acc = np.zeros((128, 128), np.float32)
        acc[0:68, :] += outs[2 * n]
        acc[61:128, :] += outs[2 * n + 1][0:67][::-1]
        res[n, 0] = acc / count
    return res


def kernel(**inputs) -> np.ndarray:
    global LAST_RESULTS, LAST_EXEC_WALL_S
    st_op = _register_st_op()
    if "nc" not in _CACHE:
        _CACHE["nc"] = _build_nc(st_op)
    nc = _CACHE["nc"]
    in_maps = _host_inputs(inputs)
    t0 = time.time()
    res = run_bass_kernel_spmd(nc, in_maps, core_ids=list(range(NCORES)))
    LAST_EXEC_WALL_S = time.time() - t0
    LAST_RESULTS = res
    outs = [res.results[c]["out"] for c in range(NCORES)]
    return _host_stitch(outs)


# revision 8
# speedup vs baseline: 14.6283x; 2.3370x over previous
"""Trainium2 Bass kernel: LISTA patch-denoiser with CBAM attention.

Sharding: 2 cores per image (4 images x 2 halves = 8 cores). Each core
owns a contiguous band of patch rows; core `2n+1` works on a vertically
flipped view of image `n` so both halves share one SPMD program (all
per-half differences — row flip, feature-order i-reversal, conv-kernel
row flip, row masks — are absorbed into the per-core input data).

Transfer-minimized design: each core receives ONE packed f32 tensor
(~1.7 MB: weights + raw 71x128 half-image). Unfold runs on-device via
overlapping strided DMAs; the final overlap-add (fold) runs on-device
via shifted-lhsT PSUM-accumulating matmuls, so the per-core output is a
[68,128] partial image instead of [128,7744] patches. The host only
stitches the two half-images and divides by the coverage count.

Device program per core:
  unfold (8 strided DMAs) -> 4-layer MLP -> pooling stats ->
  AllGather(pair) -> channel attention -> spatial attention (7x7 conv as
  14 banded matmuls) -> per-patch thresholds l -> 6 soft-thresholds
  (custom fused DVE op) interleaved with LISTA matmuls -> clipped
  reconstruction -> on-device fold -> [68,128] partial image out.
"""
import sys
import os
import time

sys.path.insert(0, "/opt/trn_rl_repo")

import numpy as np
import ml_dtypes
import jax

# Persistent XLA compilation cache: run_bass_kernel_spmd creates a fresh
# jax.jit per call, so without this every kernel() call re-compiles the
# identical HLO (~0.2s). With it, repeat calls deserialize from disk.
jax.config.update("jax_compilation_cache_dir", "/tmp/jax_cc_cache")
jax.config.update("jax_persistent_cache_min_compile_time_secs", 0)
jax.config.update("jax_persistent_cache_min_entry_size_bytes", 0)


class _PhaseDone(Exception):
    pass


import concourse.bass as bass
import concourse.tile as tile
from concourse import bacc, mybir, bass_isa
from concourse.bass_utils import run_bass_kernel_spmd
from concourse.dve_spec import (Spec, Src0, Src1, C0, Zero, relu, maxx,
                                select, lower, _has_src1)
from concourse.dve_uop import DveOpSpec
import concourse.dve_ops as dve_ops
import bass_rust

F32 = mybir.dt.float32
AF = mybir.ActivationFunctionType
ALU = mybir.AluOpType
AX = mybir.AxisListType
VP = bass_rust.VecI64Pair

P = 8
T = 5
RE = 121            # patch grid side (128 - 8 + 1)
NROW = 64           # local patch rows per core (owned + halo)
NPAT = NROW * RE    # 7744
GS = 4 * RE         # 484 patches per group (4 patch rows)
NG = 16
HALF_G = 8          # ISTA runs in two 8-group passes to halve z SBUF
NCORES = 8
D, H1, H2, H3, DL = 64, 512, 256, 128, 256
IMG_ROWS = 71       # local image rows needed: 64 patch rows + 7

# packed per-core input layouts: (name, partitions, cols)
# f32 section: small/precision-sensitive tensors + raw image
_LAYOUT32 = [
    ("b1t", 128, 4), ("b2t", 128, 2), ("b3t", 128, 1), ("b4t", 128, 2),
    ("ones1", 128, 1), ("invc", 128, 1), ("maskb1", 1, 64),
    ("sel8", 64, 8), ("rowm61", 128, 1), ("img", IMG_ROWS, 128),
]
# bf16 section: bulk weights (transfer halved, upconverted on device)
_LAYOUT16 = [
    ("w1t", 64, 512), ("w2t", 128, 1024), ("w3t", 128, 256),
    ("w4t", 128, 256), ("dct", 64, 256), ("dtt", 128, 128),
    ("st_", 128, 512), ("cw1", 128, 32), ("cw2", 16, 256),
    ("band", 64, 896),
]


def _mkoffs(layout):
    offs, tot = {}, 0
    for n_, p_, c_ in layout:
        offs[n_] = tot
        tot += p_ * c_
    return offs, tot


_OFFS32, _NTOT32 = _mkoffs(_LAYOUT32)
_OFFS16, _NTOT16 = _mkoffs(_LAYOUT16)

_CACHE = {}
LAST_RESULTS = None
LAST_EXEC_WALL_S = None


# --------------------------------------------------------------------------
# custom fused DVE soft-threshold:  out = sign(v) * relu(|v| - l * (1/c))
# --------------------------------------------------------------------------
def _register_st_op():
    name = "ST_SOFTTHRESH_ANT"
    for o in dve_ops.OPS:
        if o.name == name:
            return o
    r = relu(maxx(Src0, Zero - Src0) - Src1 * C0)
    body = select(Src0 >= Zero, r, Zero - r)

    def _ref(in0, in1, s0, s1, imm2):
        rr = np.maximum(np.maximum(in0, -in0) - in1 * s0, 0.0)
        return np.where(in0 >= 0, rr, -rr).astype(np.float32)

    spec = Spec(body=body, reference=_ref)
    opcode = dve_ops._CUSTOM_DVE_ROW_BASE + len(dve_ops.OPS)
    shas = {}
    for ver in ("v3", "v4"):
        s = DveOpSpec(name=name, opcode=opcode, uops=lower(spec, ver=ver),
                      rd1_en=_has_src1(spec))
        shas[ver] = s.sha(ver)
    op = dve_ops.DveOp(name, spec, subdim=False, uops_sha=shas)
    dve_ops.OPS.append(op)
    dve_ops._SUB_OPCODE_FOR_NAME[name] = opcode
    dve_ops.CUSTOM_DVE_SPECS[name] = spec
    return op


# --------------------------------------------------------------------------
# device program
# --------------------------------------------------------------------------
def _build_nc(st_op):
    phases = int(os.environ.get("ST_PHASES", "9"))
    nc = bacc.Bacc("TRN2", target_bir_lowering=False, debug=False,
                   num_devices=NCORES)

    a_pk = nc.dram_tensor("pk", [1, _NTOT32], F32, kind="ExternalInput").ap()
    a_pk16 = nc.dram_tensor("pk16", [1, _NTOT16], mybir.dt.bfloat16,
                            kind="ExternalInput").ap()
    a_out = nc.dram_tensor("out", [68, 128], F32, kind="ExternalOutput").ap()
    a_dbg = None
    if phases < 9:
        a_dbg = nc.dram_tensor("dbg", [128, 1024], F32,
                               kind="ExternalOutput").ap()

    with tile.TileContext(nc) as tc:
        import contextlib
        ctx = contextlib.ExitStack()
        try:
          with ctx:
            wp = ctx.enter_context(tc.tile_pool(name="wp", bufs=1))
            lamp = ctx.enter_context(tc.tile_pool(name="lamp", bufs=1))
            zp = ctx.enter_context(tc.tile_pool(name="zp", bufs=1))
            hp = ctx.enter_context(tc.tile_pool(name="hp", bufs=3))
            sp = ctx.enter_context(tc.tile_pool(name="sp", bufs=1))
            cb = ctx.enter_context(tc.tile_pool(name="cb", bufs=2))
            xpp = ctx.enter_context(tc.tile_pool(name="xpp", bufs=3))
            fip = ctx.enter_context(tc.tile_pool(name="fip", bufs=2))
            dp = ctx.enter_context(tc.tile_pool(name="dp", bufs=1,
                                                space="DRAM"))
            mlp_ctx = contextlib.ExitStack()
            mps1 = mlp_ctx.enter_context(tc.tile_pool(name="mps1", bufs=2,
                                                      space="PSUM"))
            mps2 = mlp_ctx.enter_context(tc.tile_pool(name="mps2", bufs=1,
                                                      space="PSUM"))
            mps34 = mlp_ctx.enter_context(tc.tile_pool(name="mps34", bufs=1,
                                                       space="PSUM"))

            stgp = ctx.enter_context(tc.tile_pool(name="stgp", bufs=2))

            # ---- load constants from the packed inputs ----
            def wtile(name):
                _, p_, c_ = next(e for e in _LAYOUT32 if e[0] == name)
                t = wp.tile([p_, c_], F32, tag=name, name=name)
                off = _OFFS32[name]
                nc.sync.dma_start(
                    t[:], a_pk[0:1, off:off + p_ * c_].rearrange(
                        "a (p c) -> (a p) c", p=p_, c=c_))
                return t

            def wtile16(name):
                _, p_, c_ = next(e for e in _LAYOUT16 if e[0] == name)
                stg = stgp.tile([128, 1024], mybir.dt.bfloat16, tag="stg",
                                name=f"stg_{name}")
                off = _OFFS16[name]
                nc.sync.dma_start(
                    stg[0:p_, 0:c_],
                    a_pk16[0:1, off:off + p_ * c_].rearrange(
                        "a (p c) -> (a p) c", p=p_, c=c_))
                t = wp.tile([p_, c_], F32, tag=name, name=name)
                nc.vector.tensor_copy(t[:], stg[0:p_, 0:c_])
                return t

            w1 = wtile16("w1t")
            b1 = wtile("b1t")

            # ---- on-device unfold: ufull[(j*8+i), r*121+v] = img[r+i, j+v]
            ufull = wp.tile([D, NPAT], F32, tag="ufull", name="ufull")
            for j in range(P):
                src = a_pk.copy()
                src.offset = _OFFS32["img"] + j
                src.ap = VP([[128, P], [128, NROW], [1, RE]])  # i, r, v
                nc.sync.dma_start(ufull[j * P:(j + 1) * P, :], src)

            def unf_v(g):
                return ufull[:, g * GS:(g + 1) * GS]

            w2 = wtile16("w2t")
            b2 = wtile("b2t")
            w3 = wtile16("w3t")
            b3 = wtile("b3t")
            w4 = wtile16("w4t")
            b4 = wtile("b4t")
            dct = wtile16("dct")
            dtt = wtile16("dtt")
            smat = wtile16("st_")
            cw1 = wtile16("cw1")
            cw2 = wtile16("cw2")
            ones1 = wtile("ones1")
            invc = wtile("invc")
            band = wtile16("band")
            maskb1 = wtile("maskb1")
            sel8 = wtile("sel8")
            rowm61 = wtile("rowm61")

            # derived on device: dcc = Dict/c, cw1s = ca_w1/(121*121),
            # maskb = broadcast of the [1,64] row-ownership mask
            dcc = wp.tile([D, DL], F32, tag="dcc", name="dcc")
            nc.scalar.activation(dcc[:], dct[:], AF.Copy,
                                 scale=invc[0:D, 0:1])
            cw1s = wp.tile([128, 32], F32, tag="cw1s", name="cw1s")
            nc.scalar.activation(cw1s[:], cw1[:], AF.Copy,
                                 scale=1.0 / float(RE * RE))
            maskb = wp.tile([128, NROW], F32, tag="maskb", name="maskb")
            nc.gpsimd.partition_broadcast(maskb[:], maskb1[:], 128)

            # selw [64,128]: zeros except selw[j*8+i, 60+i] = 1
            selw = wp.tile([D, 128], F32, tag="selw", name="selw")
            nc.gpsimd.memset(selw[:], 0.0)
            nc.sync.dma_start(selw[:, 60:68], sel8[:])

            rowsum = [sp.tile([128, NROW], F32, tag=f"rsum{m}", name=f"rsum{m}")
                      for m in range(2)]
            rowmax = [sp.tile([128, NROW], F32, tag=f"rmax{m}", name=f"rmax{m}")
                      for m in range(2)]

            lam_t = [[None] * NG, [None] * NG]

            if phases <= 0 and a_dbg is not None:
                nc.sync.dma_start(a_dbg[0:D, 0:GS], unf_v(0))
                nc.sync.dma_start(a_dbg[0:D, GS:2 * GS], unf_v(15))
                raise _PhaseDone()

            # =========================== MLP ===========================
            for g in range(NG):
                ps2 = [mps2.tile([128, GS], F32, tag=f"ps2_{m}", name=f"ps2_{m}")
                       for m in range(2)]
                for kk in range(4):
                    ps1 = mps1.tile([128, GS], F32, tag="ps1", name="ps1")
                    nc.tensor.matmul(ps1[:], w1[:, kk * 128:(kk + 1) * 128],
                                     unf_v(g), start=True, stop=True)
                    h1k = hp.tile([128, GS], F32, tag="h1k", name="h1k")
                    if kk % 2 == 0:
                        nc.scalar.activation(h1k[:], ps1[:], AF.Relu,
                                             bias=b1[:, kk:kk + 1])
                    else:
                        nc.vector.tensor_scalar(h1k[:], ps1[:],
                                                b1[:, kk:kk + 1], 0.0,
                                                ALU.add, ALU.max)
                    for m in range(2):
                        o = kk * 2 * H3 + m * 128
                        nc.tensor.matmul(ps2[m][:], w2[:, o:o + 128],
                                         h1k[:], start=(kk == 0),
                                         stop=(kk == 3))
                h2t = []
                for m in range(2):
                    h2m = hp.tile([128, GS], F32, tag=f"h2_{m}", name=f"h2_{m}")
                    nc.scalar.activation(h2m[:], ps2[m][:], AF.Relu,
                                         bias=b2[:, m:m + 1])
                    h2t.append(h2m)
                ps3 = mps34.tile([128, GS], F32, tag="ps3", name="ps3", bufs=2)
                for kk in range(2):
                    nc.tensor.matmul(ps3[:], w3[:, kk * 128:(kk + 1) * 128],
                                     h2t[kk][:], start=(kk == 0),
                                     stop=(kk == 1))
                h3t = hp.tile([128, GS], F32, tag="h3", name="h3")
                nc.scalar.activation(h3t[:], ps3[:], AF.Relu, bias=b3[:, 0:1])
                for m in range(2):
                    ps4 = mps34.tile([128, GS], F32, tag=f"ps4_{m}", name=f"ps4_{m}")
                    nc.tensor.matmul(ps4[:], w4[:, m * 128:(m + 1) * 128],
                                     h3t[:], start=True, stop=True)
                    lam = lamp.tile([128, GS], F32, tag=f"lam{m}_{g}", name=f"lam{m}_{g}")
                    for r in range(4):
                        rsl = slice(r * RE, (r + 1) * RE)
                        nc.scalar.activation(
                            lam[:, rsl], ps4[:, rsl], AF.Identity,
                            bias=b4[:, m:m + 1],
                            accum_out=rowsum[m][:, g * 4 + r:g * 4 + r + 1])
                    lam_t[m][g] = lam
                    ap3 = lam[:].rearrange("p (r v) -> p r v", v=RE)
                    nc.vector.tensor_reduce(
                        rowmax[m][:, g * 4:(g + 1) * 4], ap3, axis=AX.X,
                        op=ALU.max)

            mlp_ctx.close()

            if phases <= 1 and a_dbg is not None:
                nc.sync.dma_start(a_dbg[:, 0:GS], lam_t[0][0][:])
                nc.sync.dma_start(a_dbg[:, GS:2 * GS], lam_t[1][0][:])
                nc.sync.dma_start(a_dbg[:, 2 * GS:2 * GS + NROW], rowsum[0][:])
                nc.sync.dma_start(a_dbg[:, 2 * GS + NROW:2 * GS + 2 * NROW],
                                  rowmax[0][:])
                raise _PhaseDone()

            bps_ctx = contextlib.ExitStack()
            bps = bps_ctx.enter_context(tc.tile_pool(name="bps", bufs=1,
                                                     space="PSUM"))

            # ================= pooling stats + AllGather ================
            # mneg: 0 where row owned, -1e38 where not
            mneg = sp.tile([128, NROW], F32, tag="mneg", name="mneg")
            nc.vector.tensor_scalar(mneg[:], maskb[:], -1.0, 1.0e38,
                                    ALU.add, ALU.mult)
            mstat = sp.tile([128, 4], F32, tag="mstat", name="mstat")
            for m in range(2):
                t1 = sp.tile([128, NROW], F32, tag="scr1", name="scr1")
                nc.vector.tensor_tensor(t1[:], rowsum[m][:], maskb[:],
                                        op=ALU.mult)
                nc.vector.tensor_reduce(mstat[:, m:m + 1], t1[:], axis=AX.X,
                                        op=ALU.add)
                t2 = sp.tile([128, NROW], F32, tag="scr2", name="scr2")
                nc.vector.tensor_tensor(t2[:], rowmax[m][:], mneg[:],
                                        op=ALU.add)
                nc.vector.tensor_reduce(mstat[:, 2 + m:3 + m], t2[:],
                                        axis=AX.X, op=ALU.max)
            cc_in = dp.tile([128, 4], F32, name="cc_in")
            cc_out = dp.tile([1, 1024], F32, name="cc_out")
            nc.sync.dma_start(cc_in[:], mstat[:])
            nc.gpsimd.collective_compute(
                "AllGather", ALU.bypass,
                replica_groups=[[0, 1], [2, 3], [4, 5], [6, 7]],
                ins=[cc_in.opt()], outs=[cc_out.opt()])
            tg = sp.tile([128, 8], F32, tag="tg", name="tg")
            for hb in range(2):
                src = cc_out[0:1, hb * 512:(hb + 1) * 512].rearrange(
                    "a (p c) -> (a p) c", p=128, c=4)
                nc.sync.dma_start(tg[:, hb * 4:(hb + 1) * 4], src)
            st2 = sp.tile([128, 4], F32, tag="st2", name="st2")
            nc.vector.tensor_tensor(st2[:, 0:2], tg[:, 0:2], tg[:, 4:6],
                                    op=ALU.add)
            nc.vector.tensor_tensor(st2[:, 2:4], tg[:, 2:4], tg[:, 6:8],
                                    op=ALU.max)

            # ==================== channel attention =====================
            hbr = []
            for br, (wt, c0) in enumerate(((cw1s, 0), (cw1, 2))):
                psh = bps.tile([16, 1], F32, tag="psh", name="psh")
                for kk in range(2):
                    nc.tensor.matmul(psh[:], wt[:, kk * 16:(kk + 1) * 16],
                                     st2[:, c0 + kk:c0 + kk + 1],
                                     start=(kk == 0), stop=(kk == 1))
                hb_ = sp.tile([16, 1], F32, tag=f"hbr{br}", name=f"hbr{br}")
                nc.scalar.activation(hb_[:], psh[:], AF.Relu)
                hbr.append(hb_)
            ca = sp.tile([128, 2], F32, tag="ca", name="ca")
            for m in range(2):
                psca = bps.tile([128, 1], F32, tag="psca", name="psca")
                nc.tensor.matmul(psca[:], cw2[:, m * 128:(m + 1) * 128],
                                 hbr[0][:], start=True, stop=False)
                nc.tensor.matmul(psca[:], cw2[:, m * 128:(m + 1) * 128],
                                 hbr[1][:], start=False, stop=True)
                nc.scalar.activation(ca[:, m:m + 1], psca[:], AF.Sigmoid)

            if phases <= 2 and a_dbg is not None:
                nc.sync.dma_start(a_dbg[:, 0:4], mstat[:])
                nc.sync.dma_start(a_dbg[:, 4:12], tg[:])
                nc.sync.dma_start(a_dbg[:, 12:16], st2[:])
                nc.sync.dma_start(a_dbg[:, 16:18], ca[:])
                raise _PhaseDone()

            # ==================== spatial attention =====================
            mean_t = cb.tile([D, RE + 6], F32, tag="mean_t", name="mean_t")
            max_t = cb.tile([D, RE + 6], F32, tag="max_t", name="max_t")
            nc.gpsimd.memset(mean_t[:], 0.0)
            nc.gpsimd.memset(max_t[:], 0.0)
            for g in range(NG):
                for m in range(2):
                    lam = lam_t[m][g]
                    nc.scalar.activation(lam[:], lam[:], AF.Copy,
                                         scale=ca[:, m:m + 1])
                pss = bps.tile([1, GS], F32, tag="pss", name="pss")
                nc.tensor.matmul(pss[:], ones1[:], lam_t[0][g][:],
                                 start=True, stop=False)
                nc.tensor.matmul(pss[:], ones1[:], lam_t[1][g][:],
                                 start=False, stop=True)
                srs = cb.tile([1, GS], F32, tag="srs", name="srs")
                nc.scalar.activation(srs[:], pss[:], AF.Copy)
                nc.sync.dma_start(mean_t[4 * g:4 * g + 4, 3:3 + RE], srs[:])
                mx1 = cb.tile([128, GS], F32, tag="mx1", name="mx1")
                nc.vector.tensor_tensor(mx1[:], lam_t[0][g][:],
                                        lam_t[1][g][:], op=ALU.max)
                mx2 = cb.tile([128, GS], F32, tag="mx2", name="mx2")
                nc.gpsimd.partition_all_reduce(mx2[:], mx1[:], 128,
                                               bass_isa.ReduceOp.max)
                nc.sync.dma_start(max_t[4 * g:4 * g + 4, 3:3 + RE],
                                  mx2[0:1, :])
            psa = bps.tile([D, RE], F32, tag="psa", name="psa")
            idx = 0
            for dc, srct in enumerate((mean_t, max_t)):
                for dj in range(7):
                    o = (dc * 7 + dj) * 64
                    nc.tensor.matmul(psa[:], band[:, o:o + 64],
                                     srct[:, dj:dj + RE], start=(idx == 0),
                                     stop=(idx == 13))
                    idx += 1
            sa_sb = cb.tile([D, RE], F32, tag="sa_sb", name="sa_sb")
            nc.scalar.activation(sa_sb[:], psa[:], AF.Sigmoid)

            if phases <= 3 and a_dbg is not None:
                nc.sync.dma_start(a_dbg[0:D, 0:RE + 6], mean_t[:])
                nc.sync.dma_start(a_dbg[0:D, RE + 6:2 * (RE + 6)], max_t[:])
                nc.sync.dma_start(a_dbg[0:D, 2 * (RE + 6):2 * (RE + 6) + RE],
                                  sa_sb[:])
                raise _PhaseDone()

            bps_ctx.close()
            ipsv = ctx.enter_context(tc.tile_pool(name="ipsv", bufs=5,
                                                  space="PSUM"))
            ipsx = ctx.enter_context(tc.tile_pool(name="ipsx", bufs=2,
                                                  space="PSUM"))
            fps = ctx.enter_context(tc.tile_pool(name="fps", bufs=1,
                                                 space="PSUM"))
            pf = fps.tile([68, 128], F32, tag="pf", name="pf")

            # ============ thresholds l (in lam tiles) + LISTA ===========
            z_t = [[None] * NG, [None] * NG]
            for half in range(2):
                gs_ = range(half * HALF_G, (half + 1) * HALF_G)
                for g in gs_:
                    srg = cb.tile([1, GS], F32, tag="srg", name="srg")
                    nc.sync.dma_start(srg[:], sa_sb[4 * g:4 * g + 4, 0:RE])
                    sab = cb.tile([128, GS], F32, tag="sab", name="sab")
                    nc.gpsimd.partition_broadcast(sab[:], srg[:], 128)
                    for m in range(2):
                        lam = lam_t[m][g]
                        nc.vector.tensor_tensor(lam[:], lam[:], sab[:],
                                                op=ALU.mult)
                if phases <= 4 and half == 0 and a_dbg is not None:
                    nc.sync.dma_start(a_dbg[:, 0:GS], lam_t[0][0][:])
                    nc.sync.dma_start(a_dbg[:, GS:2 * GS], lam_t[1][0][:])
                    raise _PhaseDone()
                # k = 0:  z = ST(unf @ Dict, l)
                for g in gs_:
                    w = RE if g == NG - 1 else GS
                    for m in range(2):
                        psv = ipsv.tile([128, GS], F32, tag="psv", name="psv")
                        nc.tensor.matmul(psv[:, 0:w],
                                         dct[:, m * 128:(m + 1) * 128],
                                         unf_v(g)[:, 0:w], start=True,
                                         stop=True)
                        z = zp.tile([128, GS], F32, tag=f"z{m}_{g % HALF_G}", name=f"z{m}_{g % HALF_G}")
                        nc.vector._custom_dve(st_op, out=z[:, 0:w],
                                              in0=psv[:, 0:w],
                                              in1=lam_t[m][g][:, 0:w],
                                              s0=invc[:, 0:1])
                        z_t[m][g] = z
                # k = 1..T:  z = ST(z @ S + unf @ Dict/c, l)
                for k in range(T):
                    for g in gs_:
                        w = RE if g == NG - 1 else GS
                        psvs = []
                        for m in range(2):
                            psv = ipsv.tile([128, GS], F32, tag="psv", name="psv")
                            nc.tensor.matmul(
                                psv[:, 0:w], smat[:, m * 128:(m + 1) * 128],
                                z_t[0][g][:, 0:w], start=True, stop=False)
                            nc.tensor.matmul(
                                psv[:, 0:w],
                                smat[:, DL + m * 128:DL + (m + 1) * 128],
                                z_t[1][g][:, 0:w], start=False, stop=False)
                            nc.tensor.matmul(
                                psv[:, 0:w], dcc[:, m * 128:(m + 1) * 128],
                                unf_v(g)[:, 0:w], start=False, stop=True)
                            psvs.append(psv)
                        for m in range(2):
                            nc.vector._custom_dve(st_op, out=z_t[m][g][:, 0:w],
                                                  in0=psvs[m][:, 0:w],
                                                  in1=lam_t[m][g][:, 0:w],
                                                  s0=invc[:, 0:1])
                # reconstruction + on-device fold
                for g in gs_:
                    w = RE if g == NG - 1 else GS
                    nr = 1 if g == NG - 1 else 4
                    psx = ipsx.tile([D, GS], F32, tag="psx", name="psx")
                    nc.tensor.matmul(psx[:, 0:w], dtt[:, 0:D],
                                     z_t[0][g][:, 0:w],
                                     start=True, stop=False)
                    nc.tensor.matmul(psx[:, 0:w], dtt[:, D:2 * D],
                                     z_t[1][g][:, 0:w],
                                     start=False, stop=True)
                    xp = xpp.tile([D, GS], F32, tag="xp", name="xp")
                    nc.vector.tensor_scalar(xp[:, 0:w], psx[:, 0:w], 0.0, 1.0,
                                            ALU.max, ALU.min)
                    if g == NG - 1:
                        # mask patch row 60 on half-1 cores (owned by half 0)
                        nc.scalar.activation(xp[:, 0:w], xp[:, 0:w], AF.Copy,
                                             scale=rowm61[0:D, 0:1])
                    fi = fip.tile([D, 4 * 128], F32, tag="fi", name="fi")
                    nc.gpsimd.memset(fi[:, 0:nr * 128], 0.0)
                    fi3 = fi[:].rearrange("p (r c) -> p r c", c=128)
                    xp3 = xp[:, 0:nr * RE].rearrange("p (r v) -> p r v", v=RE)
                    for j in range(P):
                        nc.sync.dma_start(
                            fi3[j * P:(j + 1) * P, 0:nr, j:j + RE],
                            xp3[j * P:(j + 1) * P, :, :])
                    for r in range(nr):
                        gr = g * 4 + r
                        nc.tensor.matmul(pf[:, :],
                                         selw[:, 60 - gr:128 - gr],
                                         fi[:, r * 128:(r + 1) * 128],
                                         start=(gr == 0), stop=(gr == 60))

            outt = xpp.tile([68, 128], F32, tag="outt", name="outt")
            nc.scalar.activation(outt[:], pf[:], AF.Copy)
            nc.sync.dma_start(a_out[:, :], outt[:])

        except _PhaseDone:
            pass
    nc.compile()
    return nc


# --------------------------------------------------------------------------
# host-side data prep
# --------------------------------------------------------------------------
def _banded_conv(sa_conv, half):
    """14 banded [64,64] lhsT matrices: B[(dc,dj)][r,u] = W[dc, r-u+3, dj].
    Channel 0 (mean) carries the 1/256 mean normalization; half 1 uses the
    row-flipped kernel."""
    W = np.array(sa_conv[0], np.float32).copy()
    W[0] /= 256.0
    if half == 1:
        W = W[:, ::-1, :]
    out = np.zeros((D, 14 * 64), np.float32)
    r = np.arange(64)
    for dc in range(2):
        for dj in range(7):
            B = np.zeros((64, 64), np.float32)
            for u in range(64):
                di = r - u + 3
                ok = (di >= 0) & (di < 7)
                B[ok, u] = W[dc, di[ok], dj]
            out[:, (dc * 7 + dj) * 64:(dc * 7 + dj + 1) * 64] = B
    return out


def _half_pack(inputs, half):
    """Weight section of the packed input for one half (h0 or h1)."""
    Dict = np.asarray(inputs["Dict"], np.float32)
    cval = float(np.asarray(inputs["c"]))
    W1 = np.asarray(inputs["W1"], np.float32)
    W2 = np.asarray(inputs["W2"], np.float32)
    W3 = np.asarray(inputs["W3"], np.float32)
    W4 = np.asarray(inputs["W4"], np.float32)
    b1 = np.asarray(inputs["b1"], np.float32)
    b2 = np.asarray(inputs["b2"], np.float32)
    b3 = np.asarray(inputs["b3"], np.float32)
    b4 = np.asarray(inputs["b4"], np.float32)
    ca_w1 = np.asarray(inputs["ca_w1"], np.float32)
    ca_w2 = np.asarray(inputs["ca_w2"], np.float32)
    sa_conv = np.asarray(inputs["sa_conv"], np.float32)

    # feature order f' = j*8+i; half 1 reverses i (flipped image rows)
    perm = np.array([(i if half == 0 else 7 - i) * P + j
                     for j in range(P) for i in range(P)])
    S = (np.eye(DL, dtype=np.float32) - (Dict.T @ Dict) / cval).T
    DT = Dict.T[:, perm]
    sel8 = np.zeros((D, 8), np.float32)
    for j in range(P):
        for i in range(P):
            sel8[j * P + i, i] = 1.0
    nown = 61 - half
    mk1 = np.zeros((1, NROW), np.float32)
    mk1[:, :nown] = 1.0

    vals32 = dict(
        b1t=b1.reshape(4, 128).T,
        b2t=b2.reshape(2, 128).T,
        b3t=b3[:, None],
        b4t=b4.reshape(2, 128).T,
        ones1=np.ones((128, 1), np.float32),
        invc=np.full((128, 1), 1.0 / cval, np.float32),
        maskb1=mk1,
        sel8=sel8,
        rowm61=np.full((128, 1), 1.0 - half, np.float32),
    )
    vals16 = dict(
        w1t=W1[perm],
        w2t=np.hstack([W2[k * 128:(k + 1) * 128] for k in range(4)]),
        w3t=np.hstack([W3[k * 128:(k + 1) * 128] for k in range(2)]),
        w4t=W4,
        dct=Dict[perm],
        dtt=np.hstack([DT[k * 128:(k + 1) * 128] for k in range(2)]),
        st_=np.hstack([S[k * 128:(k + 1) * 128] for k in range(2)]),
        cw1=np.hstack([ca_w1[k * 128:(k + 1) * 128] for k in range(2)]),
        cw2=ca_w2,
        band=_banded_conv(sa_conv, half),
    )
    p32 = []
    for name, p_, c_ in _LAYOUT32:
        if name == "img":
            continue
        v = np.ascontiguousarray(vals32[name], np.float32)
        assert v.shape == (p_, c_), (name, v.shape, (p_, c_))
        p32.append(v.ravel())
    p16 = []
    for name, p_, c_ in _LAYOUT16:
        v = np.ascontiguousarray(vals16[name], np.float32)
        assert v.shape == (p_, c_), (name, v.shape, (p_, c_))
        p16.append(v.astype(ml_dtypes.bfloat16).ravel())
    return np.concatenate(p32), np.concatenate(p16)


def _host_inputs(inputs):
    x = np.asarray(inputs["x"], np.float32)
    wsec = [_half_pack(inputs, h) for h in range(2)]
    in_maps = []
    for c in range(NCORES):
        n, half = c // 2, c % 2
        if half == 0:
            img = x[n, 0, 0:IMG_ROWS, :]
        else:
            img = x[n, 0, 128 - IMG_ROWS:128, :][::-1]
        pk = np.concatenate([wsec[half][0],
                             np.ascontiguousarray(img).ravel()])
        in_maps.append({"pk": pk.reshape(1, -1),
                        "pk16": wsec[half][1].reshape(1, -1)})
    return in_maps


_COUNT = None


def _fold_count():
    global _COUNT
    if _COUNT is None:
        cnt = np.zeros((128, 128), np.float32)
        for i in range(P):
            for j in range(P):
                cnt[i:i + RE, j:j + RE] += 1.0
        _COUNT = cnt
    return _COUNT


def _host_stitch(outs):
    count = _fold_count()
    res = np.empty((4, 1, 128, 128), np.float32)
    for n in range(4):
        acc = np.zeros((128, 128), np.float32)
        acc[0:68, :] += outs[2 * n]
        acc[61:128, :] += outs[2 * n + 1][0:67][::-1]
        res[n, 0] = acc / count
    return res


def kernel(**inputs) -> np.ndarray:
    global LAST_RESULTS, LAST_EXEC_WALL_S
    st_op = _register_st_op()
    if "nc" not in _CACHE:
        _CACHE["nc"] = _build_nc(st_op)
    nc = _CACHE["nc"]
    in_maps = _host_inputs(inputs)
    t0 = time.time()
    res = run_bass_kernel_spmd(nc, in_maps, core_ids=list(range(NCORES)))
    LAST_EXEC_WALL_S = time.time() - t0
    LAST_RESULTS = res
    outs = [res.results[c]["out"] for c in range(NCORES)]
    return _host_stitch(outs)


# revision 13
# speedup vs baseline: 18.4315x; 1.2600x over previous
"""Trainium2 Bass kernel: LISTA patch-denoiser with CBAM attention.

Sharding: 2 cores per image (4 images x 2 halves = 8 cores). Each core
owns a contiguous band of patch rows; core `2n+1` works on a vertically
flipped view of image `n` so both halves share one SPMD program (all
per-half differences — row flip, feature-order i-reversal, conv-kernel
row flip, row masks — are absorbed into the per-core input data).

Transfer-minimized design: each core receives ONE packed f32 tensor
(~1.7 MB: weights + raw 71x128 half-image). Unfold runs on-device via
overlapping strided DMAs; the final overlap-add (fold) runs on-device
via shifted-lhsT PSUM-accumulating matmuls, so the per-core output is a
[68,128] partial image instead of [128,7744] patches. The host only
stitches the two half-images and divides by the coverage count.

Device program per core:
  unfold (8 strided DMAs) -> 4-layer MLP -> pooling stats ->
  AllGather(pair) -> channel attention -> spatial attention (7x7 conv as
  14 banded matmuls) -> per-patch thresholds l -> 6 soft-thresholds
  (custom fused DVE op) interleaved with LISTA matmuls -> clipped
  reconstruction -> on-device fold -> [68,128] partial image out.
"""
import sys
import os
import time

sys.path.insert(0, "/opt/trn_rl_repo")

import numpy as np
import ml_dtypes
import jax

# Persistent XLA compilation cache: run_bass_kernel_spmd creates a fresh
# jax.jit per call, so without this every kernel() call re-compiles the
# identical HLO (~0.2s). With it, repeat calls deserialize from disk.
jax.config.update("jax_compilation_cache_dir", "/tmp/jax_cc_cache")
jax.config.update("jax_persistent_cache_min_compile_time_secs", 0)
jax.config.update("jax_persistent_cache_min_entry_size_bytes", 0)


class _PhaseDone(Exception):
    pass


import concourse.bass as bass
import concourse.tile as tile
from concourse import bacc, mybir, bass_isa
from concourse.bass_utils import run_bass_kernel_spmd
from concourse.dve_spec import (Spec, Src0, Src1, C0, Zero, relu, maxx,
                                select, lower, _has_src1)
from concourse.dve_uop import DveOpSpec
import concourse.dve_ops as dve_ops
import bass_rust

F32 = mybir.dt.float32
AF = mybir.ActivationFunctionType
ALU = mybir.AluOpType
AX = mybir.AxisListType
VP = bass_rust.VecI64Pair

P = 8
T = 5
RE = 121            # patch grid side (128 - 8 + 1)
NROW = 64           # local patch rows per core (owned + halo)
NPAT = NROW * RE    # 7744
GS = 4 * RE         # 484 patches per group (4 patch rows)
NG = 16
HALF_G = 8          # ISTA runs in two 8-group passes to halve z SBUF
NCORES = 8
D, H1, H2, H3, DL = 64, 512, 256, 128, 256
IMG_ROWS = 71       # local image rows needed: 64 patch rows + 7

# packed per-core input layouts: (name, partitions, cols)
# f32 section: small/precision-sensitive tensors + dictionary + raw image.
# dcc/dtt/st_ (ISTA matrices) and the banded conv lhsT are derived on
# device from dct/wsa, so the iterated LISTA path stays full f32.
_LAYOUT32 = [
    ("b1t", 128, 4), ("b2t", 128, 2), ("b3t", 128, 1), ("b4t", 128, 2),
    ("invc", 128, 1), ("ninvc", 128, 1), ("maskb1", 1, 64),
    ("sel8", 64, 8), ("rowm61", 128, 1), ("dct", 64, 256),
    ("wsa", 1, 98), ("img", IMG_ROWS, 128),
]
# bf16 section: bulk MLP/attention weights (transfer halved, upconverted)
_LAYOUT16 = [
    ("w1t", 64, 512), ("w2t", 128, 1024), ("w3t", 128, 256),
    ("w4t", 128, 256), ("cw1", 128, 32), ("cw2", 16, 256),
]


def _mkoffs(layout):
    offs, tot = {}, 0
    for n_, p_, c_ in layout:
        offs[n_] = tot
        tot += p_ * c_
    return offs, tot


_OFFS32, _NTOT32 = _mkoffs(_LAYOUT32)
_OFFS16, _NTOT16 = _mkoffs(_LAYOUT16)

_CACHE = {}
LAST_RESULTS = None
LAST_EXEC_WALL_S = None


# --------------------------------------------------------------------------
# custom fused DVE soft-threshold:  out = sign(v) * relu(|v| - l * (1/c))
# --------------------------------------------------------------------------
def _register_st_op():
    name = "ST_SOFTTHRESH_ANT"
    for o in dve_ops.OPS:
        if o.name == name:
            return o
    r = relu(maxx(Src0, Zero - Src0) - Src1 * C0)
    body = select(Src0 >= Zero, r, Zero - r)

    def _ref(in0, in1, s0, s1, imm2):
        rr = np.maximum(np.maximum(in0, -in0) - in1 * s0, 0.0)
        return np.where(in0 >= 0, rr, -rr).astype(np.float32)

    spec = Spec(body=body, reference=_ref)
    opcode = dve_ops._CUSTOM_DVE_ROW_BASE + len(dve_ops.OPS)
    shas = {}
    for ver in ("v3", "v4"):
        s = DveOpSpec(name=name, opcode=opcode, uops=lower(spec, ver=ver),
                      rd1_en=_has_src1(spec))
        shas[ver] = s.sha(ver)
    op = dve_ops.DveOp(name, spec, subdim=False, uops_sha=shas)
    dve_ops.OPS.append(op)
    dve_ops._SUB_OPCODE_FOR_NAME[name] = opcode
    dve_ops.CUSTOM_DVE_SPECS[name] = spec
    return op


# --------------------------------------------------------------------------
# device program
# --------------------------------------------------------------------------
def _build_nc(st_op):
    phases = int(os.environ.get("ST_PHASES", "9"))
    nc = bacc.Bacc("TRN2", target_bir_lowering=False, debug=False,
                   num_devices=NCORES)

    a_pk = nc.dram_tensor("pk", [1, _NTOT32], F32, kind="ExternalInput").ap()
    a_pk16 = nc.dram_tensor("pk16", [1, _NTOT16], mybir.dt.bfloat16,
                            kind="ExternalInput").ap()
    a_out = nc.dram_tensor("out", [68, 128], F32, kind="ExternalOutput").ap()
    a_dbg = None
    if phases < 9:
        a_dbg = nc.dram_tensor("dbg", [128, 1024], F32,
                               kind="ExternalOutput").ap()

    with tile.TileContext(nc) as tc:
        import contextlib
        ctx = contextlib.ExitStack()
        try:
          with ctx:
            wp = ctx.enter_context(tc.tile_pool(name="wp", bufs=1))
            lamp = ctx.enter_context(tc.tile_pool(name="lamp", bufs=1))
            zp = ctx.enter_context(tc.tile_pool(name="zp", bufs=1))
            hp = ctx.enter_context(tc.tile_pool(name="hp", bufs=3))
            sp = ctx.enter_context(tc.tile_pool(name="sp", bufs=1))
            cb = ctx.enter_context(tc.tile_pool(name="cb", bufs=2))
            xpp = ctx.enter_context(tc.tile_pool(name="xpp", bufs=3))
            fip = ctx.enter_context(tc.tile_pool(name="fip", bufs=2))
            dp = ctx.enter_context(tc.tile_pool(name="dp", bufs=1,
                                                space="DRAM"))
            stgp = ctx.enter_context(tc.tile_pool(name="stgp", bufs=2))

            # ---- load constants from the packed inputs ----
            def wtile(name):
                _, p_, c_ = next(e for e in _LAYOUT32 if e[0] == name)
                t = wp.tile([p_, c_], F32, tag=name, name=name)
                off = _OFFS32[name]
                nc.sync.dma_start(
                    t[:], a_pk[0:1, off:off + p_ * c_].rearrange(
                        "a (p c) -> (a p) c", p=p_, c=c_))
                return t

            def wtile16(name):
                _, p_, c_ = next(e for e in _LAYOUT16 if e[0] == name)
                stg = stgp.tile([128, 1024], mybir.dt.bfloat16, tag="stg",
                                name=f"stg_{name}")
                off = _OFFS16[name]
                nc.sync.dma_start(
                    stg[0:p_, 0:c_],
                    a_pk16[0:1, off:off + p_ * c_].rearrange(
                        "a (p c) -> (a p) c", p=p_, c=c_))
                t = wp.tile([p_, c_], F32, tag=name, name=name)
                nc.vector.tensor_copy(t[:], stg[0:p_, 0:c_])
                return t

            w1 = wtile16("w1t")
            b1 = wtile("b1t")

            # ---- on-device unfold: ufull[(j*8+i), r*121+v] = img[r+i, j+v]
            ufull = wp.tile([D, NPAT], F32, tag="ufull", name="ufull")
            for j in range(P):
                src = a_pk.copy()
                src.offset = _OFFS32["img"] + j
                src.ap = VP([[128, P], [128, NROW], [1, RE]])  # i, r, v
                nc.sync.dma_start(ufull[j * P:(j + 1) * P, :], src)

            def unf_v(g):
                return ufull[:, g * GS:(g + 1) * GS]

            w2 = wtile16("w2t")
            b2 = wtile("b2t")
            w3 = wtile16("w3t")
            b3 = wtile("b3t")
            w4 = wtile16("w4t")
            b4 = wtile("b4t")
            dct = wtile("dct")
            cw1 = wtile16("cw1")
            cw2 = wtile16("cw2")
            invc = wtile("invc")
            ninvc = wtile("ninvc")
            maskb1 = wtile("maskb1")
            sel8 = wtile("sel8")
            rowm61 = wtile("rowm61")
            wsa1 = wtile("wsa")

            ones1 = wp.tile([128, 1], F32, tag="ones1", name="ones1")
            nc.gpsimd.memset(ones1[:], 1.0)

            # ---- derived constants ----
            # id64 (identity) via affine_select iota compare
            id64 = wp.tile([D, D], F32, tag="id64", name="id64")
            nc.gpsimd.affine_select(
                id64[:], ones1[0:D, 0:1].to_broadcast([D, D]),
                pattern=[[-1, D]], compare_op=ALU.is_equal, fill=0.0,
                base=0, channel_multiplier=1)
            # banded 7x7-conv lhsT blocks: B[(dc,dj)][r,u] = W[dc,r-u+3,dj]
            wb = wp.tile([D, 98], F32, tag="wb", name="wb")
            nc.gpsimd.partition_broadcast(wb[:], wsa1[:], D)
            band = wp.tile([D, 14 * 64], F32, tag="band", name="band")
            tmpd = wp.tile([D, D], F32, tag="tmpd", name="tmpd")
            for dc in range(2):
                for dj in range(7):
                    blk = band[:, (dc * 7 + dj) * 64:(dc * 7 + dj + 1) * 64]
                    for di in range(7):
                        col = dc * 49 + di * 7 + dj
                        dst = blk if di == 0 else tmpd[:]
                        nc.gpsimd.affine_select(
                            dst, wb[:, col:col + 1].to_broadcast([D, D]),
                            pattern=[[-1, D]], compare_op=ALU.is_equal,
                            fill=0.0, base=3 - di, channel_multiplier=1)
                        if di > 0:
                            nc.vector.tensor_tensor(blk, blk, tmpd[:],
                                                    op=ALU.add)
            # smat block k = I_shift(k) - (Dict^T Dict)[k-rows]/c  and
            # dtt = Dict.T blocks, both from dct via PE
            smat = wp.tile([128, 2 * DL], F32, tag="st_", name="st_")
            dtt = wp.tile([128, 2 * D], F32, tag="dtt", name="dtt")
            ish = wp.tile([128, DL], F32, tag="ish", name="ish")
            gctx = contextlib.ExitStack()
            gps = gctx.enter_context(tc.tile_pool(name="gps", bufs=1,
                                                  space="PSUM"))
            for k in range(2):
                psG = gps.tile([128, DL], F32, tag="psG", name="psG")
                nc.tensor.matmul(psG[:], dct[:, k * 128:(k + 1) * 128],
                                 dct[:], start=True, stop=True)
                sl = smat[:, k * DL:(k + 1) * DL]
                nc.scalar.activation(sl, psG[:], AF.Copy,
                                     scale=ninvc[:, 0:1])
                nc.gpsimd.affine_select(
                    ish[:], ones1[:, 0:1].to_broadcast([128, DL]),
                    pattern=[[-1, DL]], compare_op=ALU.is_equal, fill=0.0,
                    base=k * 128, channel_multiplier=1)
                nc.vector.tensor_tensor(sl, sl, ish[:], op=ALU.add)
                psT = gps.tile([128, D], F32, tag="psT", name="psT")
                nc.tensor.matmul(psT[:], dct[:, k * 128:(k + 1) * 128],
                                 id64[:], start=True, stop=True)
                nc.scalar.activation(dtt[:, k * D:(k + 1) * D], psT[:],
                                     AF.Copy)
            gctx.close()

            # dcc = Dict/c, cw1s = ca_w1/(121*121), maskb broadcast
            dcc = wp.tile([D, DL], F32, tag="dcc", name="dcc")
            nc.scalar.activation(dcc[:], dct[:], AF.Copy,
                                 scale=invc[0:D, 0:1])
            cw1s = wp.tile([128, 32], F32, tag="cw1s", name="cw1s")
            nc.scalar.activation(cw1s[:], cw1[:], AF.Copy,
                                 scale=1.0 / float(RE * RE))
            maskb = wp.tile([128, NROW], F32, tag="maskb", name="maskb")
            nc.gpsimd.partition_broadcast(maskb[:], maskb1[:], 128)

            mlp_ctx = contextlib.ExitStack()
            mps1 = mlp_ctx.enter_context(tc.tile_pool(name="mps1", bufs=2,
                                                      space="PSUM"))
            mps2 = mlp_ctx.enter_context(tc.tile_pool(name="mps2", bufs=1,
                                                      space="PSUM"))
            mps34 = mlp_ctx.enter_context(tc.tile_pool(name="mps34", bufs=1,
                                                       space="PSUM"))

            # selw [64,128]: zeros except selw[j*8+i, 60+i] = 1
            selw = wp.tile([D, 128], F32, tag="selw", name="selw")
            nc.gpsimd.memset(selw[:], 0.0)
            nc.sync.dma_start(selw[:, 60:68], sel8[:])

            rowsum = [sp.tile([128, NROW], F32, tag=f"rsum{m}", name=f"rsum{m}")
                      for m in range(2)]
            rowmax = [sp.tile([128, NROW], F32, tag=f"rmax{m}", name=f"rmax{m}")
                      for m in range(2)]

            lam_t = [[None] * NG, [None] * NG]

            if phases <= 0 and a_dbg is not None:
                nc.sync.dma_start(a_dbg[0:D, 0:GS], unf_v(0))
                nc.sync.dma_start(a_dbg[0:D, GS:2 * GS], unf_v(15))
                raise _PhaseDone()

            # =========================== MLP ===========================
            for g in range(NG):
                ps2 = [mps2.tile([128, GS], F32, tag=f"ps2_{m}", name=f"ps2_{m}")
                       for m in range(2)]
                for kk in range(4):
                    ps1 = mps1.tile([128, GS], F32, tag="ps1", name="ps1")
                    nc.tensor.matmul(ps1[:], w1[:, kk * 128:(kk + 1) * 128],
                                     unf_v(g), start=True, stop=True)
                    h1k = hp.tile([128, GS], F32, tag="h1k", name="h1k")
                    if kk % 2 == 0:
                        nc.scalar.activation(h1k[:], ps1[:], AF.Relu,
                                             bias=b1[:, kk:kk + 1])
                    else:
                        nc.vector.tensor_scalar(h1k[:], ps1[:],
                                                b1[:, kk:kk + 1], 0.0,
                                                ALU.add, ALU.max)
                    for m in range(2):
                        o = kk * 2 * H3 + m * 128
                        nc.tensor.matmul(ps2[m][:], w2[:, o:o + 128],
                                         h1k[:], start=(kk == 0),
                                         stop=(kk == 3))
                h2t = []
                for m in range(2):
                    h2m = hp.tile([128, GS], F32, tag=f"h2_{m}", name=f"h2_{m}")
                    nc.scalar.activation(h2m[:], ps2[m][:], AF.Relu,
                                         bias=b2[:, m:m + 1])
                    h2t.append(h2m)
                ps3 = mps34.tile([128, GS], F32, tag="ps3", name="ps3", bufs=2)
                for kk in range(2):
                    nc.tensor.matmul(ps3[:], w3[:, kk * 128:(kk + 1) * 128],
                                     h2t[kk][:], start=(kk == 0),
                                     stop=(kk == 1))
                h3t = hp.tile([128, GS], F32, tag="h3", name="h3")
                nc.scalar.activation(h3t[:], ps3[:], AF.Relu, bias=b3[:, 0:1])
                for m in range(2):
                    ps4 = mps34.tile([128, GS], F32, tag=f"ps4_{m}", name=f"ps4_{m}")
                    nc.tensor.matmul(ps4[:], w4[:, m * 128:(m + 1) * 128],
                                     h3t[:], start=True, stop=True)
                    lam = lamp.tile([128, GS], F32, tag=f"lam{m}_{g}", name=f"lam{m}_{g}")
                    for r in range(4):
                        rsl = slice(r * RE, (r + 1) * RE)
                        nc.scalar.activation(
                            lam[:, rsl], ps4[:, rsl], AF.Identity,
                            bias=b4[:, m:m + 1],
                            accum_out=rowsum[m][:, g * 4 + r:g * 4 + r + 1])
                    lam_t[m][g] = lam
                    ap3 = lam[:].rearrange("p (r v) -> p r v", v=RE)
                    nc.vector.tensor_reduce(
                        rowmax[m][:, g * 4:(g + 1) * 4], ap3, axis=AX.X,
                        op=ALU.max)

            mlp_ctx.close()

            if phases <= 1 and a_dbg is not None:
                nc.sync.dma_start(a_dbg[:, 0:GS], lam_t[0][0][:])
                nc.sync.dma_start(a_dbg[:, GS:2 * GS], lam_t[1][0][:])
                nc.sync.dma_start(a_dbg[:, 2 * GS:2 * GS + NROW], rowsum[0][:])
                nc.sync.dma_start(a_dbg[:, 2 * GS + NROW:2 * GS + 2 * NROW],
                                  rowmax[0][:])
                raise _PhaseDone()

            bps_ctx = contextlib.ExitStack()
            bps = bps_ctx.enter_context(tc.tile_pool(name="bps", bufs=1,
                                                     space="PSUM"))

            # ================= pooling stats + AllGather ================
            # mneg: 0 where row owned, -1e38 where not
            mneg = sp.tile([128, NROW], F32, tag="mneg", name="mneg")
            nc.vector.tensor_scalar(mneg[:], maskb[:], -1.0, 1.0e38,
                                    ALU.add, ALU.mult)
            mstat = sp.tile([128, 4], F32, tag="mstat", name="mstat")
            for m in range(2):
                t1 = sp.tile([128, NROW], F32, tag="scr1", name="scr1")
                nc.vector.tensor_tensor(t1[:], rowsum[m][:], maskb[:],
                                        op=ALU.mult)
                nc.vector.tensor_reduce(mstat[:, m:m + 1], t1[:], axis=AX.X,
                                        op=ALU.add)
                t2 = sp.tile([128, NROW], F32, tag="scr2", name="scr2")
                nc.vector.tensor_tensor(t2[:], rowmax[m][:], mneg[:],
                                        op=ALU.add)
                nc.vector.tensor_reduce(mstat[:, 2 + m:3 + m], t2[:],
                                        axis=AX.X, op=ALU.max)
            cc_in = dp.tile([128, 4], F32, name="cc_in")
            cc_out = dp.tile([1, 1024], F32, name="cc_out")
            nc.sync.dma_start(cc_in[:], mstat[:])
            nc.gpsimd.collective_compute(
                "AllGather", ALU.bypass,
                replica_groups=[[0, 1], [2, 3], [4, 5], [6, 7]],
                ins=[cc_in.opt()], outs=[cc_out.opt()])
            tg = sp.tile([128, 8], F32, tag="tg", name="tg")
            for hb in range(2):
                src = cc_out[0:1, hb * 512:(hb + 1) * 512].rearrange(
                    "a (p c) -> (a p) c", p=128, c=4)
                nc.sync.dma_start(tg[:, hb * 4:(hb + 1) * 4], src)
            st2 = sp.tile([128, 4], F32, tag="st2", name="st2")
            nc.vector.tensor_tensor(st2[:, 0:2], tg[:, 0:2], tg[:, 4:6],
                                    op=ALU.add)
            nc.vector.tensor_tensor(st2[:, 2:4], tg[:, 2:4], tg[:, 6:8],
                                    op=ALU.max)

            # ==================== channel attention =====================
            hbr = []
            for br, (wt, c0) in enumerate(((cw1s, 0), (cw1, 2))):
                psh = bps.tile([16, 1], F32, tag="psh", name="psh")
                for kk in range(2):
                    nc.tensor.matmul(psh[:], wt[:, kk * 16:(kk + 1) * 16],
                                     st2[:, c0 + kk:c0 + kk + 1],
                                     start=(kk == 0), stop=(kk == 1))
                hb_ = sp.tile([16, 1], F32, tag=f"hbr{br}", name=f"hbr{br}")
                nc.scalar.activation(hb_[:], psh[:], AF.Relu)
                hbr.append(hb_)
            ca = sp.tile([128, 2], F32, tag="ca", name="ca")
            for m in range(2):
                psca = bps.tile([128, 1], F32, tag="psca", name="psca")
                nc.tensor.matmul(psca[:], cw2[:, m * 128:(m + 1) * 128],
                                 hbr[0][:], start=True, stop=False)
                nc.tensor.matmul(psca[:], cw2[:, m * 128:(m + 1) * 128],
                                 hbr[1][:], start=False, stop=True)
                nc.scalar.activation(ca[:, m:m + 1], psca[:], AF.Sigmoid)

            if phases <= 2 and a_dbg is not None:
                nc.sync.dma_start(a_dbg[:, 0:4], mstat[:])
                nc.sync.dma_start(a_dbg[:, 4:12], tg[:])
                nc.sync.dma_start(a_dbg[:, 12:16], st2[:])
                nc.sync.dma_start(a_dbg[:, 16:18], ca[:])
                raise _PhaseDone()

            # ==================== spatial attention =====================
            mean_t = cb.tile([D, RE + 6], F32, tag="mean_t", name="mean_t")
            max_t = cb.tile([D, RE + 6], F32, tag="max_t", name="max_t")
            nc.gpsimd.memset(mean_t[:], 0.0)
            nc.gpsimd.memset(max_t[:], 0.0)
            for g in range(NG):
                for m in range(2):
                    lam = lam_t[m][g]
                    nc.scalar.activation(lam[:], lam[:], AF.Copy,
                                         scale=ca[:, m:m + 1])
                pss = bps.tile([1, GS], F32, tag="pss", name="pss")
                nc.tensor.matmul(pss[:], ones1[:], lam_t[0][g][:],
                                 start=True, stop=False)
                nc.tensor.matmul(pss[:], ones1[:], lam_t[1][g][:],
                                 start=False, stop=True)
                srs = cb.tile([1, GS], F32, tag="srs", name="srs")
                nc.scalar.activation(srs[:], pss[:], AF.Copy)
                nc.sync.dma_start(mean_t[4 * g:4 * g + 4, 3:3 + RE], srs[:])
                mx1 = cb.tile([128, GS], F32, tag="mx1", name="mx1")
                nc.vector.tensor_tensor(mx1[:], lam_t[0][g][:],
                                        lam_t[1][g][:], op=ALU.max)
                mx2 = cb.tile([128, GS], F32, tag="mx2", name="mx2")
                nc.gpsimd.partition_all_reduce(mx2[:], mx1[:], 128,
                                               bass_isa.ReduceOp.max)
                nc.sync.dma_start(max_t[4 * g:4 * g + 4, 3:3 + RE],
                                  mx2[0:1, :])
            psa = bps.tile([D, RE], F32, tag="psa", name="psa")
            idx = 0
            for dc, srct in enumerate((mean_t, max_t)):
                for dj in range(7):
                    o = (dc * 7 + dj) * 64
                    nc.tensor.matmul(psa[:], band[:, o:o + 64],
                                     srct[:, dj:dj + RE], start=(idx == 0),
                                     stop=(idx == 13))
                    idx += 1
            sa_sb = cb.tile([D, RE], F32, tag="sa_sb", name="sa_sb")
            nc.scalar.activation(sa_sb[:], psa[:], AF.Sigmoid)

            if phases <= 3 and a_dbg is not None:
                nc.sync.dma_start(a_dbg[0:D, 0:RE + 6], mean_t[:])
                nc.sync.dma_start(a_dbg[0:D, RE + 6:2 * (RE + 6)], max_t[:])
                nc.sync.dma_start(a_dbg[0:D, 2 * (RE + 6):2 * (RE + 6) + RE],
                                  sa_sb[:])
                raise _PhaseDone()

            bps_ctx.close()
            ipsv = ctx.enter_context(tc.tile_pool(name="ipsv", bufs=5,
                                                  space="PSUM"))
            ipsx = ctx.enter_context(tc.tile_pool(name="ipsx", bufs=2,
                                                  space="PSUM"))
            fps = ctx.enter_context(tc.tile_pool(name="fps", bufs=1,
                                                 space="PSUM"))
            pf = fps.tile([68, 128], F32, tag="pf", name="pf")

            # ============ thresholds l (in lam tiles) + LISTA ===========
            z_t = [[None] * NG, [None] * NG]
            for half in range(2):
                gs_ = range(half * HALF_G, (half + 1) * HALF_G)
                for g in gs_:
                    srg = cb.tile([1, GS], F32, tag="srg", name="srg")
                    nc.sync.dma_start(srg[:], sa_sb[4 * g:4 * g + 4, 0:RE])
                    sab = cb.tile([128, GS], F32, tag="sab", name="sab")
                    nc.gpsimd.partition_broadcast(sab[:], srg[:], 128)
                    for m in range(2):
                        lam = lam_t[m][g]
                        nc.vector.tensor_tensor(lam[:], lam[:], sab[:],
                                                op=ALU.mult)
                if phases <= 4 and half == 0 and a_dbg is not None:
                    nc.sync.dma_start(a_dbg[:, 0:GS], lam_t[0][0][:])
                    nc.sync.dma_start(a_dbg[:, GS:2 * GS], lam_t[1][0][:])
                    raise _PhaseDone()
                # k = 0:  z = ST(unf @ Dict, l)
                for g in gs_:
                    w = RE if g == NG - 1 else GS
                    for m in range(2):
                        psv = ipsv.tile([128, GS], F32, tag="psv", name="psv")
                        nc.tensor.matmul(psv[:, 0:w],
                                         dct[:, m * 128:(m + 1) * 128],
                                         unf_v(g)[:, 0:w], start=True,
                                         stop=True)
                        z = zp.tile([128, GS], F32, tag=f"z{m}_{g % HALF_G}", name=f"z{m}_{g % HALF_G}")
                        nc.vector._custom_dve(st_op, out=z[:, 0:w],
                                              in0=psv[:, 0:w],
                                              in1=lam_t[m][g][:, 0:w],
                                              s0=invc[:, 0:1])
                        z_t[m][g] = z
                # k = 1..T:  z = ST(z @ S + unf @ Dict/c, l)
                for k in range(T):
                    for g in gs_:
                        w = RE if g == NG - 1 else GS
                        psvs = []
                        for m in range(2):
                            psv = ipsv.tile([128, GS], F32, tag="psv", name="psv")
                            nc.tensor.matmul(
                                psv[:, 0:w], smat[:, m * 128:(m + 1) * 128],
                                z_t[0][g][:, 0:w], start=True, stop=False)
                            nc.tensor.matmul(
                                psv[:, 0:w],
                                smat[:, DL + m * 128:DL + (m + 1) * 128],
                                z_t[1][g][:, 0:w], start=False, stop=False)
                            nc.tensor.matmul(
                                psv[:, 0:w], dcc[:, m * 128:(m + 1) * 128],
                                unf_v(g)[:, 0:w], start=False, stop=True)
                            psvs.append(psv)
                        for m in range(2):
                            nc.vector._custom_dve(st_op, out=z_t[m][g][:, 0:w],
                                                  in0=psvs[m][:, 0:w],
                                                  in1=lam_t[m][g][:, 0:w],
                                                  s0=invc[:, 0:1])
                # reconstruction + on-device fold
                for g in gs_:
                    w = RE if g == NG - 1 else GS
                    nr = 1 if g == NG - 1 else 4
                    psx = ipsx.tile([D, GS], F32, tag="psx", name="psx")
                    nc.tensor.matmul(psx[:, 0:w], dtt[:, 0:D],
                                     z_t[0][g][:, 0:w],
                                     start=True, stop=False)
                    nc.tensor.matmul(psx[:, 0:w], dtt[:, D:2 * D],
                                     z_t[1][g][:, 0:w],
                                     start=False, stop=True)
                    xp = xpp.tile([D, GS], F32, tag="xp", name="xp")
                    nc.vector.tensor_scalar(xp[:, 0:w], psx[:, 0:w], 0.0, 1.0,
                                            ALU.max, ALU.min)
                    if g == NG - 1:
                        # mask patch row 60 on half-1 cores (owned by half 0)
                        nc.scalar.activation(xp[:, 0:w], xp[:, 0:w], AF.Copy,
                                             scale=rowm61[0:D, 0:1])
                    fi = fip.tile([D, 4 * 128], F32, tag="fi", name="fi")
                    nc.gpsimd.memset(fi[:, 0:nr * 128], 0.0)
                    fi3 = fi[:].rearrange("p (r c) -> p r c", c=128)
                    xp3 = xp[:, 0:nr * RE].rearrange("p (r v) -> p r v", v=RE)
                    for j in range(P):
                        nc.sync.dma_start(
                            fi3[j * P:(j + 1) * P, 0:nr, j:j + RE],
                            xp3[j * P:(j + 1) * P, :, :])
                    for r in range(nr):
                        gr = g * 4 + r
                        nc.tensor.matmul(pf[:, :],
                                         selw[:, 60 - gr:128 - gr],
                                         fi[:, r * 128:(r + 1) * 128],
                                         start=(gr == 0), stop=(gr == 60))

            outt = xpp.tile([68, 128], F32, tag="outt", name="outt")
            nc.scalar.activation(outt[:], pf[:], AF.Copy)
            nc.sync.dma_start(a_out[:, :], outt[:])

        except _PhaseDone:
            pass
    nc.compile()
    return nc


# --------------------------------------------------------------------------
# host-side data prep
# --------------------------------------------------------------------------
def _half_pack(inputs, half):
    """Weight section of the packed input for one half (h0 or h1)."""
    Dict = np.asarray(inputs["Dict"], np.float32)
    cval = float(np.asarray(inputs["c"]))
    W1 = np.asarray(inputs["W1"], np.float32)
    W2 = np.asarray(inputs["W2"], np.float32)
    W3 = np.asarray(inputs["W3"], np.float32)
    W4 = np.asarray(inputs["W4"], np.float32)
    b1 = np.asarray(inputs["b1"], np.float32)
    b2 = np.asarray(inputs["b2"], np.float32)
    b3 = np.asarray(inputs["b3"], np.float32)
    b4 = np.asarray(inputs["b4"], np.float32)
    ca_w1 = np.asarray(inputs["ca_w1"], np.float32)
    ca_w2 = np.asarray(inputs["ca_w2"], np.float32)
    sa_conv = np.asarray(inputs["sa_conv"], np.float32)

    # feature order f' = j*8+i; half 1 reverses i (flipped image rows)
    perm = np.array([(i if half == 0 else 7 - i) * P + j
                     for j in range(P) for i in range(P)])
    sel8 = np.zeros((D, 8), np.float32)
    for j in range(P):
        for i in range(P):
            sel8[j * P + i, i] = 1.0
    nown = 61 - half
    mk1 = np.zeros((1, NROW), np.float32)
    mk1[:, :nown] = 1.0
    # 7x7 conv kernel, [dc, di, dj] order; mean-channel carries the 1/256
    # normalization; half 1 uses the row(di)-flipped kernel
    Wc = np.array(sa_conv[0], np.float32).copy()
    Wc[0] /= 256.0
    if half == 1:
        Wc = Wc[:, ::-1, :]

    vals32 = dict(
        b1t=b1.reshape(4, 128).T,
        b2t=b2.reshape(2, 128).T,
        b3t=b3[:, None],
        b4t=b4.reshape(2, 128).T,
        invc=np.full((128, 1), 1.0 / cval, np.float32),
        ninvc=np.full((128, 1), -1.0 / cval, np.float32),
        maskb1=mk1,
        sel8=sel8,
        rowm61=np.full((128, 1), 1.0 - half, np.float32),
        dct=Dict[perm],
        wsa=Wc.reshape(1, 98),
    )
    vals16 = dict(
        w1t=W1[perm],
        w2t=np.hstack([W2[k * 128:(k + 1) * 128] for k in range(4)]),
        w3t=np.hstack([W3[k * 128:(k + 1) * 128] for k in range(2)]),
        w4t=W4,
        cw1=np.hstack([ca_w1[k * 128:(k + 1) * 128] for k in range(2)]),
        cw2=ca_w2,
    )
    p32 = []
    for name, p_, c_ in _LAYOUT32:
        if name == "img":
            continue
        v = np.ascontiguousarray(vals32[name], np.float32)
        assert v.shape == (p_, c_), (name, v.shape, (p_, c_))
        p32.append(v.ravel())
    p16 = []
    for name, p_, c_ in _LAYOUT16:
        v = np.ascontiguousarray(vals16[name], np.float32)
        assert v.shape == (p_, c_), (name, v.shape, (p_, c_))
        p16.append(v.astype(ml_dtypes.bfloat16).ravel())
    return np.concatenate(p32), np.concatenate(p16)


def _host_inputs(inputs):
    x = np.asarray(inputs["x"], np.float32)
    wsec = [_half_pack(inputs, h) for h in range(2)]
    in_maps = []
    for c in range(NCORES):
        n, half = c // 2, c % 2
        if half == 0:
            img = x[n, 0, 0:IMG_ROWS, :]
        else:
            img = x[n, 0, 128 - IMG_ROWS:128, :][::-1]
        pk = np.concatenate([wsec[half][0],
                             np.ascontiguousarray(img).ravel()])
        in_maps.append({"pk": pk.reshape(1, -1),
                        "pk16": wsec[half][1].reshape(1, -1)})
    return in_maps


_COUNT = None


def _fold_count():
    global _COUNT
    if _COUNT is None:
        cnt = np.zeros((128, 128), np.float32)
        for i in range(P):
            for j in range(P):
                cnt[i:i + RE, j:j + RE] += 1.0
        _COUNT = cnt
    return _COUNT


def _host_stitch(outs):
    count = _fold_count()
    res = np.empty((4, 1, 128, 128), np.float32)
    for n in range(4):
        acc = np.zeros((128, 128), np.float32)
        acc[0:68, :] += outs[2 * n]
        acc[61:128, :] += outs[2 * n + 1][0:67][::-1]
        res[n, 0] = acc / count
    return res


def kernel(**inputs) -> np.ndarray:
    global LAST_RESULTS, LAST_EXEC_WALL_S
    st_op = _register_st_op()
    if "nc" not in _CACHE:
        _CACHE["nc"] = _build_nc(st_op)
    nc = _CACHE["nc"]
    in_maps = _host_inputs(inputs)
    t0 = time.time()
    res = run_bass_kernel_spmd(nc, in_maps, core_ids=list(range(NCORES)))
    LAST_EXEC_WALL_S = time.time() - t0
    LAST_RESULTS = res
    outs = [res.results[c]["out"] for c in range(NCORES)]
    return _host_stitch(outs)


# revision 17
# speedup vs baseline: 18.8804x; 1.0244x over previous
"""Trainium2 Bass kernel: LISTA patch-denoiser with CBAM attention.

Sharding: 2 cores per image (4 images x 2 halves = 8 cores). Each core
owns a contiguous band of patch rows; core `2n+1` works on a vertically
flipped view of image `n` so both halves share one SPMD program (all
per-half differences — row flip, feature-order i-reversal, conv-kernel
row flip, row masks — are absorbed into the per-core input data).

Transfer-minimized design: each core receives ONE packed f32 tensor
(~1.7 MB: weights + raw 71x128 half-image). Unfold runs on-device via
overlapping strided DMAs; the final overlap-add (fold) runs on-device
via shifted-lhsT PSUM-accumulating matmuls, so the per-core output is a
[68,128] partial image instead of [128,7744] patches. The host only
stitches the two half-images and divides by the coverage count.

Device program per core:
  unfold (8 strided DMAs) -> 4-layer MLP -> pooling stats ->
  AllGather(pair) -> channel attention -> spatial attention (7x7 conv as
  14 banded matmuls) -> per-patch thresholds l -> 6 soft-thresholds
  (custom fused DVE op) interleaved with LISTA matmuls -> clipped
  reconstruction -> on-device fold -> [68,128] partial image out.
"""
import sys
import os
import time

sys.path.insert(0, "/opt/trn_rl_repo")

import numpy as np
import ml_dtypes
import jax

# Persistent XLA compilation cache: run_bass_kernel_spmd creates a fresh
# jax.jit per call, so without this every kernel() call re-compiles the
# identical HLO (~0.2s). With it, repeat calls deserialize from disk.
jax.config.update("jax_compilation_cache_dir", "/tmp/jax_cc_cache")
jax.config.update("jax_persistent_cache_min_compile_time_secs", 0)
jax.config.update("jax_persistent_cache_min_entry_size_bytes", 0)


class _PhaseDone(Exception):
    pass


import concourse.bass as bass
import concourse.tile as tile
from concourse import bacc, mybir, bass_isa
from concourse.bass_utils import run_bass_kernel_spmd
from concourse.dve_spec import (Spec, Src0, Src1, C0, Zero, relu, maxx,
                                select, lower, _has_src1)
from concourse.dve_uop import DveOpSpec
import concourse.dve_ops as dve_ops
import bass_rust

F32 = mybir.dt.float32
AF = mybir.ActivationFunctionType
ALU = mybir.AluOpType
AX = mybir.AxisListType
VP = bass_rust.VecI64Pair

P = 8
T = 5
RE = 121            # patch grid side (128 - 8 + 1)
NROW = 64           # local patch rows per core (owned + halo)
NPAT = NROW * RE    # 7744
GS = 4 * RE         # 484 patches per group (4 patch rows)
NG = 16
HALF_G = 8          # ISTA runs in two 8-group passes to halve z SBUF
NCORES = 8
D, H1, H2, H3, DL = 64, 512, 256, 128, 256
IMG_ROWS = 71       # local image rows needed: 64 patch rows + 7

# packed per-core input layouts: (name, partitions, cols)
# f32 section: small/precision-sensitive tensors + dictionary + raw image.
# dcc/dtt/st_ (ISTA matrices) and the banded conv lhsT are derived on
# device from dct/wsa, so the iterated LISTA path stays full f32.
_LAYOUT32 = [
    ("b1t", 128, 4), ("b2t", 128, 2), ("b3t", 128, 1), ("b4t", 128, 2),
    ("invc", 128, 1), ("ninvc", 128, 1), ("maskb1", 1, 64),
    ("sel8", 64, 8), ("rowm61", 128, 1), ("dct", 64, 256),
    ("wsa", 1, 98), ("img", IMG_ROWS, 128),
]
# bf16 section: bulk MLP/attention weights (transfer halved, upconverted)
_LAYOUT16 = [
    ("w1t", 64, 512), ("w2t", 128, 1024), ("w3t", 128, 256),
    ("w4t", 128, 256), ("cw1", 128, 32), ("cw2", 16, 256),
]


def _mkoffs(layout):
    offs, tot = {}, 0
    for n_, p_, c_ in layout:
        offs[n_] = tot
        tot += p_ * c_
    return offs, tot


_OFFS32, _NTOT32 = _mkoffs(_LAYOUT32)
_OFFS16, _NTOT16 = _mkoffs(_LAYOUT16)
# single combined f32 tensor: f32 section, then bf16 section (2 per slot)
_NTOTC = _NTOT32 + _NTOT16 // 2

_CACHE = {}
LAST_RESULTS = None
LAST_EXEC_WALL_S = None


# --------------------------------------------------------------------------
# custom fused DVE soft-threshold:  out = sign(v) * relu(|v| - l * (1/c))
# --------------------------------------------------------------------------
def _register_st_op():
    name = "ST_SOFTTHRESH_ANT"
    for o in dve_ops.OPS:
        if o.name == name:
            return o
    r = relu(maxx(Src0, Zero - Src0) - Src1 * C0)
    body = select(Src0 >= Zero, r, Zero - r)

    def _ref(in0, in1, s0, s1, imm2):
        rr = np.maximum(np.maximum(in0, -in0) - in1 * s0, 0.0)
        return np.where(in0 >= 0, rr, -rr).astype(np.float32)

    spec = Spec(body=body, reference=_ref)
    opcode = dve_ops._CUSTOM_DVE_ROW_BASE + len(dve_ops.OPS)
    shas = {}
    for ver in ("v3", "v4"):
        s = DveOpSpec(name=name, opcode=opcode, uops=lower(spec, ver=ver),
                      rd1_en=_has_src1(spec))
        shas[ver] = s.sha(ver)
    op = dve_ops.DveOp(name, spec, subdim=False, uops_sha=shas)
    dve_ops.OPS.append(op)
    dve_ops._SUB_OPCODE_FOR_NAME[name] = opcode
    dve_ops.CUSTOM_DVE_SPECS[name] = spec
    return op


# --------------------------------------------------------------------------
# device program
# --------------------------------------------------------------------------
def _build_nc(st_op):
    phases = int(os.environ.get("ST_PHASES", "9"))
    nc = bacc.Bacc("TRN2", target_bir_lowering=False, debug=False,
                   num_devices=NCORES)

    a_pk = nc.dram_tensor("pk", [1, _NTOTC], F32, kind="ExternalInput").ap()
    a_pk16 = a_pk.bitcast(mybir.dt.bfloat16)
    a_out = nc.dram_tensor("out", [68, 128], F32, kind="ExternalOutput").ap()
    a_dbg = None
    if phases < 9:
        a_dbg = nc.dram_tensor("dbg", [128, 1024], F32,
                               kind="ExternalOutput").ap()

    with tile.TileContext(nc) as tc:
        import contextlib
        ctx = contextlib.ExitStack()
        try:
          with ctx:
            wp = ctx.enter_context(tc.tile_pool(name="wp", bufs=1))
            lamp = ctx.enter_context(tc.tile_pool(name="lamp", bufs=1))
            zp = ctx.enter_context(tc.tile_pool(name="zp", bufs=1))
            hp = ctx.enter_context(tc.tile_pool(name="hp", bufs=3))
            sp = ctx.enter_context(tc.tile_pool(name="sp", bufs=1))
            cb = ctx.enter_context(tc.tile_pool(name="cb", bufs=2))
            xpp = ctx.enter_context(tc.tile_pool(name="xpp", bufs=3))
            fip = ctx.enter_context(tc.tile_pool(name="fip", bufs=2))
            dp = ctx.enter_context(tc.tile_pool(name="dp", bufs=1,
                                                space="DRAM"))
            stgp = ctx.enter_context(tc.tile_pool(name="stgp", bufs=2))

            # ---- load constants from the packed inputs ----
            def wtile(name):
                _, p_, c_ = next(e for e in _LAYOUT32 if e[0] == name)
                t = wp.tile([p_, c_], F32, tag=name, name=name)
                off = _OFFS32[name]
                nc.sync.dma_start(
                    t[:], a_pk[0:1, off:off + p_ * c_].rearrange(
                        "a (p c) -> (a p) c", p=p_, c=c_))
                return t

            def wtile16(name):
                _, p_, c_ = next(e for e in _LAYOUT16 if e[0] == name)
                stg = stgp.tile([128, 1024], mybir.dt.bfloat16, tag="stg",
                                name=f"stg_{name}")
                off = 2 * _NTOT32 + _OFFS16[name]
                nc.sync.dma_start(
                    stg[0:p_, 0:c_],
                    a_pk16[0:1, off:off + p_ * c_].rearrange(
                        "a (p c) -> (a p) c", p=p_, c=c_))
                t = wp.tile([p_, c_], F32, tag=name, name=name)
                nc.vector.tensor_copy(t[:], stg[0:p_, 0:c_])
                return t

            w1 = wtile16("w1t")
            b1 = wtile("b1t")

            # ---- on-device unfold: ufull[(j*8+i), r*121+v] = img[r+i, j+v]
            ufull = wp.tile([D, NPAT], F32, tag="ufull", name="ufull")
            for j in range(P):
                src = a_pk.copy()
                src.offset = _OFFS32["img"] + j
                src.ap = VP([[128, P], [128, NROW], [1, RE]])  # i, r, v
                nc.sync.dma_start(ufull[j * P:(j + 1) * P, :], src)

            def unf_v(g):
                return ufull[:, g * GS:(g + 1) * GS]

            w2 = wtile16("w2t")
            b2 = wtile("b2t")
            w3 = wtile16("w3t")
            b3 = wtile("b3t")
            w4 = wtile16("w4t")
            b4 = wtile("b4t")
            dct = wtile("dct")
            cw1 = wtile16("cw1")
            cw2 = wtile16("cw2")
            invc = wtile("invc")
            ninvc = wtile("ninvc")
            maskb1 = wtile("maskb1")
            sel8 = wtile("sel8")
            rowm61 = wtile("rowm61")
            wsa1 = wtile("wsa")

            ones1 = wp.tile([128, 1], F32, tag="ones1", name="ones1")
            nc.gpsimd.memset(ones1[:], 1.0)

            # ---- derived constants ----
            # id64 (identity) via affine_select iota compare
            id64 = wp.tile([D, D], F32, tag="id64", name="id64")
            nc.gpsimd.affine_select(
                id64[:], ones1[0:D, 0:1].to_broadcast([D, D]),
                pattern=[[-1, D]], compare_op=ALU.is_equal, fill=0.0,
                base=0, channel_multiplier=1)
            # banded 7x7-conv lhsT blocks: B[(dc,dj)][r,u] = W[dc,r-u+3,dj]
            wb = wp.tile([D, 98], F32, tag="wb", name="wb")
            nc.gpsimd.partition_broadcast(wb[:], wsa1[:], D)
            band = wp.tile([D, 14 * 64], F32, tag="band", name="band")
            tmpd = wp.tile([D, D], F32, tag="tmpd", name="tmpd")
            for dc in range(2):
                for dj in range(7):
                    blk = band[:, (dc * 7 + dj) * 64:(dc * 7 + dj + 1) * 64]
                    for di in range(7):
                        col = dc * 49 + di * 7 + dj
                        dst = blk if di == 0 else tmpd[:]
                        nc.gpsimd.affine_select(
                            dst, wb[:, col:col + 1].to_broadcast([D, D]),
                            pattern=[[-1, D]], compare_op=ALU.is_equal,
                            fill=0.0, base=3 - di, channel_multiplier=1)
                        if di > 0:
                            nc.vector.tensor_tensor(blk, blk, tmpd[:],
                                                    op=ALU.add)
            # smat block k = I_shift(k) - (Dict^T Dict)[k-rows]/c  and
            # dtt = Dict.T blocks, both from dct via PE
            smat = wp.tile([128, 2 * DL], F32, tag="st_", name="st_")
            dtt = wp.tile([128, 2 * D], F32, tag="dtt", name="dtt")
            ish = wp.tile([128, DL], F32, tag="ish", name="ish")
            gctx = contextlib.ExitStack()
            gps = gctx.enter_context(tc.tile_pool(name="gps", bufs=1,
                                                  space="PSUM"))
            for k in range(2):
                psG = gps.tile([128, DL], F32, tag="psG", name="psG")
                nc.tensor.matmul(psG[:], dct[:, k * 128:(k + 1) * 128],
                                 dct[:], start=True, stop=True)
                sl = smat[:, k * DL:(k + 1) * DL]
                nc.scalar.activation(sl, psG[:], AF.Copy,
                                     scale=ninvc[:, 0:1])
                nc.gpsimd.affine_select(
                    ish[:], ones1[:, 0:1].to_broadcast([128, DL]),
                    pattern=[[-1, DL]], compare_op=ALU.is_equal, fill=0.0,
                    base=k * 128, channel_multiplier=1)
                nc.vector.tensor_tensor(sl, sl, ish[:], op=ALU.add)
                psT = gps.tile([128, D], F32, tag="psT", name="psT")
                nc.tensor.matmul(psT[:], dct[:, k * 128:(k + 1) * 128],
                                 id64[:], start=True, stop=True)
                nc.scalar.activation(dtt[:, k * D:(k + 1) * D], psT[:],
                                     AF.Copy)
            gctx.close()

            # dcc = Dict/c, cw1s = ca_w1/(121*121), maskb broadcast
            dcc = wp.tile([D, DL], F32, tag="dcc", name="dcc")
            nc.scalar.activation(dcc[:], dct[:], AF.Copy,
                                 scale=invc[0:D, 0:1])
            cw1s = wp.tile([128, 32], F32, tag="cw1s", name="cw1s")
            nc.scalar.activation(cw1s[:], cw1[:], AF.Copy,
                                 scale=1.0 / float(RE * RE))
            maskb = wp.tile([128, NROW], F32, tag="maskb", name="maskb")
            nc.gpsimd.partition_broadcast(maskb[:], maskb1[:], 128)

            mlp_ctx = contextlib.ExitStack()
            mps1 = mlp_ctx.enter_context(tc.tile_pool(name="mps1", bufs=2,
                                                      space="PSUM"))
            mps2 = mlp_ctx.enter_context(tc.tile_pool(name="mps2", bufs=1,
                                                      space="PSUM"))
            mps34 = mlp_ctx.enter_context(tc.tile_pool(name="mps34", bufs=1,
                                                       space="PSUM"))

            # selw [64,128]: zeros except selw[j*8+i, 60+i] = 1
            selw = wp.tile([D, 128], F32, tag="selw", name="selw")
            nc.gpsimd.memset(selw[:], 0.0)
            nc.sync.dma_start(selw[:, 60:68], sel8[:])

            rowsum = [sp.tile([128, NROW], F32, tag=f"rsum{m}", name=f"rsum{m}")
                      for m in range(2)]
            rowmax = [sp.tile([128, NROW], F32, tag=f"rmax{m}", name=f"rmax{m}")
                      for m in range(2)]

            lam_t = [[None] * NG, [None] * NG]

            if phases <= 0 and a_dbg is not None:
                nc.sync.dma_start(a_dbg[0:D, 0:GS], unf_v(0))
                nc.sync.dma_start(a_dbg[0:D, GS:2 * GS], unf_v(15))
                raise _PhaseDone()

            # =========================== MLP ===========================
            for g in range(NG):
                ps2 = [mps2.tile([128, GS], F32, tag=f"ps2_{m}", name=f"ps2_{m}")
                       for m in range(2)]
                for kk in range(4):
                    ps1 = mps1.tile([128, GS], F32, tag="ps1", name="ps1")
                    nc.tensor.matmul(ps1[:], w1[:, kk * 128:(kk + 1) * 128],
                                     unf_v(g), start=True, stop=True)
                    h1k = hp.tile([128, GS], F32, tag="h1k", name="h1k")
                    if kk % 2 == 0:
                        nc.scalar.activation(h1k[:], ps1[:], AF.Relu,
                                             bias=b1[:, kk:kk + 1])
                    else:
                        nc.vector.tensor_scalar(h1k[:], ps1[:],
                                                b1[:, kk:kk + 1], 0.0,
                                                ALU.add, ALU.max)
                    for m in range(2):
                        o = kk * 2 * H3 + m * 128
                        nc.tensor.matmul(ps2[m][:], w2[:, o:o + 128],
                                         h1k[:], start=(kk == 0),
                                         stop=(kk == 3))
                h2t = []
                for m in range(2):
                    h2m = hp.tile([128, GS], F32, tag=f"h2_{m}", name=f"h2_{m}")
                    nc.scalar.activation(h2m[:], ps2[m][:], AF.Relu,
                                         bias=b2[:, m:m + 1])
                    h2t.append(h2m)
                ps3 = mps34.tile([128, GS], F32, tag="ps3", name="ps3", bufs=2)
                for kk in range(2):
                    nc.tensor.matmul(ps3[:], w3[:, kk * 128:(kk + 1) * 128],
                                     h2t[kk][:], start=(kk == 0),
                                     stop=(kk == 1))
                h3t = hp.tile([128, GS], F32, tag="h3", name="h3")
                nc.scalar.activation(h3t[:], ps3[:], AF.Relu, bias=b3[:, 0:1])
                for m in range(2):
                    ps4 = mps34.tile([128, GS], F32, tag=f"ps4_{m}", name=f"ps4_{m}")
                    nc.tensor.matmul(ps4[:], w4[:, m * 128:(m + 1) * 128],
                                     h3t[:], start=True, stop=True)
                    lam = lamp.tile([128, GS], F32, tag=f"lam{m}_{g}", name=f"lam{m}_{g}")
                    for r in range(4):
                        rsl = slice(r * RE, (r + 1) * RE)
                        nc.scalar.activation(
                            lam[:, rsl], ps4[:, rsl], AF.Identity,
                            bias=b4[:, m:m + 1],
                            accum_out=rowsum[m][:, g * 4 + r:g * 4 + r + 1])
                    lam_t[m][g] = lam
                    ap3 = lam[:].rearrange("p (r v) -> p r v", v=RE)
                    nc.vector.tensor_reduce(
                        rowmax[m][:, g * 4:(g + 1) * 4], ap3, axis=AX.X,
                        op=ALU.max)

            mlp_ctx.close()

            if phases <= 1 and a_dbg is not None:
                nc.sync.dma_start(a_dbg[:, 0:GS], lam_t[0][0][:])
                nc.sync.dma_start(a_dbg[:, GS:2 * GS], lam_t[1][0][:])
                nc.sync.dma_start(a_dbg[:, 2 * GS:2 * GS + NROW], rowsum[0][:])
                nc.sync.dma_start(a_dbg[:, 2 * GS + NROW:2 * GS + 2 * NROW],
                                  rowmax[0][:])
                raise _PhaseDone()

            bps_ctx = contextlib.ExitStack()
            bps = bps_ctx.enter_context(tc.tile_pool(name="bps", bufs=1,
                                                     space="PSUM"))

            # ================= pooling stats + AllGather ================
            # mneg: 0 where row owned, -1e38 where not
            mneg = sp.tile([128, NROW], F32, tag="mneg", name="mneg")
            nc.vector.tensor_scalar(mneg[:], maskb[:], -1.0, 1.0e38,
                                    ALU.add, ALU.mult)
            mstat = sp.tile([128, 4], F32, tag="mstat", name="mstat")
            for m in range(2):
                t1 = sp.tile([128, NROW], F32, tag="scr1", name="scr1")
                nc.vector.tensor_tensor(t1[:], rowsum[m][:], maskb[:],
                                        op=ALU.mult)
                nc.vector.tensor_reduce(mstat[:, m:m + 1], t1[:], axis=AX.X,
                                        op=ALU.add)
                t2 = sp.tile([128, NROW], F32, tag="scr2", name="scr2")
                nc.vector.tensor_tensor(t2[:], rowmax[m][:], mneg[:],
                                        op=ALU.add)
                nc.vector.tensor_reduce(mstat[:, 2 + m:3 + m], t2[:],
                                        axis=AX.X, op=ALU.max)
            cc_in = dp.tile([128, 4], F32, name="cc_in")
            cc_out = dp.tile([1, 1024], F32, name="cc_out")
            nc.sync.dma_start(cc_in[:], mstat[:])
            nc.gpsimd.collective_compute(
                "AllGather", ALU.bypass,
                replica_groups=[[0, 1], [2, 3], [4, 5], [6, 7]],
                ins=[cc_in.opt()], outs=[cc_out.opt()])
            tg = sp.tile([128, 8], F32, tag="tg", name="tg")
            for hb in range(2):
                src = cc_out[0:1, hb * 512:(hb + 1) * 512].rearrange(
                    "a (p c) -> (a p) c", p=128, c=4)
                nc.sync.dma_start(tg[:, hb * 4:(hb + 1) * 4], src)
            st2 = sp.tile([128, 4], F32, tag="st2", name="st2")
            nc.vector.tensor_tensor(st2[:, 0:2], tg[:, 0:2], tg[:, 4:6],
                                    op=ALU.add)
            nc.vector.tensor_tensor(st2[:, 2:4], tg[:, 2:4], tg[:, 6:8],
                                    op=ALU.max)

            # ==================== channel attention =====================
            hbr = []
            for br, (wt, c0) in enumerate(((cw1s, 0), (cw1, 2))):
                psh = bps.tile([16, 1], F32, tag="psh", name="psh")
                for kk in range(2):
                    nc.tensor.matmul(psh[:], wt[:, kk * 16:(kk + 1) * 16],
                                     st2[:, c0 + kk:c0 + kk + 1],
                                     start=(kk == 0), stop=(kk == 1))
                hb_ = sp.tile([16, 1], F32, tag=f"hbr{br}", name=f"hbr{br}")
                nc.scalar.activation(hb_[:], psh[:], AF.Relu)
                hbr.append(hb_)
            ca = sp.tile([128, 2], F32, tag="ca", name="ca")
            for m in range(2):
                psca = bps.tile([128, 1], F32, tag="psca", name="psca")
                nc.tensor.matmul(psca[:], cw2[:, m * 128:(m + 1) * 128],
                                 hbr[0][:], start=True, stop=False)
                nc.tensor.matmul(psca[:], cw2[:, m * 128:(m + 1) * 128],
                                 hbr[1][:], start=False, stop=True)
                nc.scalar.activation(ca[:, m:m + 1], psca[:], AF.Sigmoid)

            if phases <= 2 and a_dbg is not None:
                nc.sync.dma_start(a_dbg[:, 0:4], mstat[:])
                nc.sync.dma_start(a_dbg[:, 4:12], tg[:])
                nc.sync.dma_start(a_dbg[:, 12:16], st2[:])
                nc.sync.dma_start(a_dbg[:, 16:18], ca[:])
                raise _PhaseDone()

            # ==================== spatial attention =====================
            mean_t = cb.tile([D, RE + 6], F32, tag="mean_t", name="mean_t")
            max_t = cb.tile([D, RE + 6], F32, tag="max_t", name="max_t")
            nc.gpsimd.memset(mean_t[:], 0.0)
            nc.gpsimd.memset(max_t[:], 0.0)
            for g in range(NG):
                for m in range(2):
                    lam = lam_t[m][g]
                    nc.scalar.activation(lam[:], lam[:], AF.Copy,
                                         scale=ca[:, m:m + 1])
                pss = bps.tile([1, GS], F32, tag="pss", name="pss")
                nc.tensor.matmul(pss[:], ones1[:], lam_t[0][g][:],
                                 start=True, stop=False)
                nc.tensor.matmul(pss[:], ones1[:], lam_t[1][g][:],
                                 start=False, stop=True)
                srs = cb.tile([1, GS], F32, tag="srs", name="srs")
                nc.scalar.activation(srs[:], pss[:], AF.Copy)
                nc.sync.dma_start(mean_t[4 * g:4 * g + 4, 3:3 + RE], srs[:])
                mx1 = cb.tile([128, GS], F32, tag="mx1", name="mx1")
                nc.vector.tensor_tensor(mx1[:], lam_t[0][g][:],
                                        lam_t[1][g][:], op=ALU.max)
                mx2 = cb.tile([128, GS], F32, tag="mx2", name="mx2")
                nc.gpsimd.partition_all_reduce(mx2[:], mx1[:], 128,
                                               bass_isa.ReduceOp.max)
                nc.sync.dma_start(max_t[4 * g:4 * g + 4, 3:3 + RE],
                                  mx2[0:1, :])
            psa = bps.tile([D, RE], F32, tag="psa", name="psa")
            idx = 0
            for dc, srct in enumerate((mean_t, max_t)):
                for dj in range(7):
                    o = (dc * 7 + dj) * 64
                    nc.tensor.matmul(psa[:], band[:, o:o + 64],
                                     srct[:, dj:dj + RE], start=(idx == 0),
                                     stop=(idx == 13))
                    idx += 1
            sa_sb = cb.tile([D, RE], F32, tag="sa_sb", name="sa_sb")
            nc.scalar.activation(sa_sb[:], psa[:], AF.Sigmoid)

            if phases <= 3 and a_dbg is not None:
                nc.sync.dma_start(a_dbg[0:D, 0:RE + 6], mean_t[:])
                nc.sync.dma_start(a_dbg[0:D, RE + 6:2 * (RE + 6)], max_t[:])
                nc.sync.dma_start(a_dbg[0:D, 2 * (RE + 6):2 * (RE + 6) + RE],
                                  sa_sb[:])
                raise _PhaseDone()

            bps_ctx.close()
            ipsv = ctx.enter_context(tc.tile_pool(name="ipsv", bufs=5,
                                                  space="PSUM"))
            ipsx = ctx.enter_context(tc.tile_pool(name="ipsx", bufs=2,
                                                  space="PSUM"))
            fps = ctx.enter_context(tc.tile_pool(name="fps", bufs=1,
                                                 space="PSUM"))
            pf = fps.tile([68, 128], F32, tag="pf", name="pf")

            # ============ thresholds l (in lam tiles) + LISTA ===========
            z_t = [[None] * NG, [None] * NG]
            for half in range(2):
                gs_ = range(half * HALF_G, (half + 1) * HALF_G)
                for g in gs_:
                    srg = cb.tile([1, GS], F32, tag="srg", name="srg")
                    nc.sync.dma_start(srg[:], sa_sb[4 * g:4 * g + 4, 0:RE])
                    sab = cb.tile([128, GS], F32, tag="sab", name="sab")
                    nc.gpsimd.partition_broadcast(sab[:], srg[:], 128)
                    for m in range(2):
                        lam = lam_t[m][g]
                        nc.vector.tensor_tensor(lam[:], lam[:], sab[:],
                                                op=ALU.mult)
                if phases <= 4 and half == 0 and a_dbg is not None:
                    nc.sync.dma_start(a_dbg[:, 0:GS], lam_t[0][0][:])
                    nc.sync.dma_start(a_dbg[:, GS:2 * GS], lam_t[1][0][:])
                    raise _PhaseDone()
                # k = 0:  z = ST(unf @ Dict, l)
                for g in gs_:
                    w = RE if g == NG - 1 else GS
                    for m in range(2):
                        psv = ipsv.tile([128, GS], F32, tag="psv", name="psv")
                        nc.tensor.matmul(psv[:, 0:w],
                                         dct[:, m * 128:(m + 1) * 128],
                                         unf_v(g)[:, 0:w], start=True,
                                         stop=True)
                        z = zp.tile([128, GS], F32, tag=f"z{m}_{g % HALF_G}", name=f"z{m}_{g % HALF_G}")
                        nc.vector._custom_dve(st_op, out=z[:, 0:w],
                                              in0=psv[:, 0:w],
                                              in1=lam_t[m][g][:, 0:w],
                                              s0=invc[:, 0:1])
                        z_t[m][g] = z
                # k = 1..T:  z = ST(z @ S + unf @ Dict/c, l)
                for k in range(T):
                    for g in gs_:
                        w = RE if g == NG - 1 else GS
                        psvs = []
                        for m in range(2):
                            psv = ipsv.tile([128, GS], F32, tag="psv", name="psv")
                            nc.tensor.matmul(
                                psv[:, 0:w], smat[:, m * 128:(m + 1) * 128],
                                z_t[0][g][:, 0:w], start=True, stop=False)
                            nc.tensor.matmul(
                                psv[:, 0:w],
                                smat[:, DL + m * 128:DL + (m + 1) * 128],
                                z_t[1][g][:, 0:w], start=False, stop=False)
                            nc.tensor.matmul(
                                psv[:, 0:w], dcc[:, m * 128:(m + 1) * 128],
                                unf_v(g)[:, 0:w], start=False, stop=True)
                            psvs.append(psv)
                        for m in range(2):
                            nc.vector._custom_dve(st_op, out=z_t[m][g][:, 0:w],
                                                  in0=psvs[m][:, 0:w],
                                                  in1=lam_t[m][g][:, 0:w],
                                                  s0=invc[:, 0:1])
                # reconstruction + on-device fold
                for g in gs_:
                    w = RE if g == NG - 1 else GS
                    nr = 1 if g == NG - 1 else 4
                    psx = ipsx.tile([D, GS], F32, tag="psx", name="psx")
                    nc.tensor.matmul(psx[:, 0:w], dtt[:, 0:D],
                                     z_t[0][g][:, 0:w],
                                     start=True, stop=False)
                    nc.tensor.matmul(psx[:, 0:w], dtt[:, D:2 * D],
                                     z_t[1][g][:, 0:w],
                                     start=False, stop=True)
                    xp = xpp.tile([D, GS], F32, tag="xp", name="xp")
                    nc.vector.tensor_scalar(xp[:, 0:w], psx[:, 0:w], 0.0, 1.0,
                                            ALU.max, ALU.min)
                    if g == NG - 1:
                        # mask patch row 60 on half-1 cores (owned by half 0)
                        nc.scalar.activation(xp[:, 0:w], xp[:, 0:w], AF.Copy,
                                             scale=rowm61[0:D, 0:1])
                    fi = fip.tile([D, 4 * 128], F32, tag="fi", name="fi")
                    nc.gpsimd.memset(fi[:, 0:nr * 128], 0.0)
                    fi3 = fi[:].rearrange("p (r c) -> p r c", c=128)
                    xp3 = xp[:, 0:nr * RE].rearrange("p (r v) -> p r v", v=RE)
                    for j in range(P):
                        nc.sync.dma_start(
                            fi3[j * P:(j + 1) * P, 0:nr, j:j + RE],
                            xp3[j * P:(j + 1) * P, :, :])
                    for r in range(nr):
                        gr = g * 4 + r
                        nc.tensor.matmul(pf[:, :],
                                         selw[:, 60 - gr:128 - gr],
                                         fi[:, r * 128:(r + 1) * 128],
                                         start=(gr == 0), stop=(gr == 60))

            outt = xpp.tile([68, 128], F32, tag="outt", name="outt")
            nc.scalar.activation(outt[:], pf[:], AF.Copy)
            nc.sync.dma_start(a_out[:, :], outt[:])

        except _PhaseDone:
            pass
    nc.compile()
    return nc


# --------------------------------------------------------------------------
# host-side data prep
# --------------------------------------------------------------------------
def _half_pack(inputs, half):
    """Weight section of the packed input for one half (h0 or h1)."""
    Dict = np.asarray(inputs["Dict"], np.float32)
    cval = float(np.asarray(inputs["c"]))
    W1 = np.asarray(inputs["W1"], np.float32)
    W2 = np.asarray(inputs["W2"], np.float32)
    W3 = np.asarray(inputs["W3"], np.float32)
    W4 = np.asarray(inputs["W4"], np.float32)
    b1 = np.asarray(inputs["b1"], np.float32)
    b2 = np.asarray(inputs["b2"], np.float32)
    b3 = np.asarray(inputs["b3"], np.float32)
    b4 = np.asarray(inputs["b4"], np.float32)
    ca_w1 = np.asarray(inputs["ca_w1"], np.float32)
    ca_w2 = np.asarray(inputs["ca_w2"], np.float32)
    sa_conv = np.asarray(inputs["sa_conv"], np.float32)

    # feature order f' = j*8+i; half 1 reverses i (flipped image rows)
    perm = np.array([(i if half == 0 else 7 - i) * P + j
                     for j in range(P) for i in range(P)])
    sel8 = np.zeros((D, 8), np.float32)
    for j in range(P):
        for i in range(P):
            sel8[j * P + i, i] = 1.0
    nown = 61 - half
    mk1 = np.zeros((1, NROW), np.float32)
    mk1[:, :nown] = 1.0
    # 7x7 conv kernel, [dc, di, dj] order; mean-channel carries the 1/256
    # normalization; half 1 uses the row(di)-flipped kernel
    Wc = np.array(sa_conv[0], np.float32).copy()
    Wc[0] /= 256.0
    if half == 1:
        Wc = Wc[:, ::-1, :]

    vals32 = dict(
        b1t=b1.reshape(4, 128).T,
        b2t=b2.reshape(2, 128).T,
        b3t=b3[:, None],
        b4t=b4.reshape(2, 128).T,
        invc=np.full((128, 1), 1.0 / cval, np.float32),
        ninvc=np.full((128, 1), -1.0 / cval, np.float32),
        maskb1=mk1,
        sel8=sel8,
        rowm61=np.full((128, 1), 1.0 - half, np.float32),
        dct=Dict[perm],
        wsa=Wc.reshape(1, 98),
    )
    vals16 = dict(
        w1t=W1[perm],
        w2t=np.hstack([W2[k * 128:(k + 1) * 128] for k in range(4)]),
        w3t=np.hstack([W3[k * 128:(k + 1) * 128] for k in range(2)]),
        w4t=W4,
        cw1=np.hstack([ca_w1[k * 128:(k + 1) * 128] for k in range(2)]),
        cw2=ca_w2,
    )
    p32 = []
    for name, p_, c_ in _LAYOUT32:
        if name == "img":
            continue
        v = np.ascontiguousarray(vals32[name], np.float32)
        assert v.shape == (p_, c_), (name, v.shape, (p_, c_))
        p32.append(v.ravel())
    p16 = []
    for name, p_, c_ in _LAYOUT16:
        v = np.ascontiguousarray(vals16[name], np.float32)
        assert v.shape == (p_, c_), (name, v.shape, (p_, c_))
        p16.append(v.astype(ml_dtypes.bfloat16).ravel())
    return np.concatenate(p32), np.concatenate(p16)


def _host_inputs(inputs):
    x = np.asarray(inputs["x"], np.float32)
    wsec = [_half_pack(inputs, h) for h in range(2)]
    in_maps = []
    for c in range(NCORES):
        n, half = c // 2, c % 2
        if half == 0:
            img = x[n, 0, 0:IMG_ROWS, :]
        else:
            img = x[n, 0, 128 - IMG_ROWS:128, :][::-1]
        pk = np.empty((1, _NTOTC), np.float32)
        w32, w16 = wsec[half]
        n32 = w32.shape[0]
        pk[0, :n32] = w32
        pk[0, n32:_NTOT32] = img.ravel()
        pk[0, _NTOT32:].view(ml_dtypes.bfloat16)[:] = w16
        in_maps.append({"pk": pk})
    return in_maps


_COUNT = None


def _fold_count():
    global _COUNT
    if _COUNT is None:
        cnt = np.zeros((128, 128), np.float32)
        for i in range(P):
            for j in range(P):
                cnt[i:i + RE, j:j + RE] += 1.0
        _COUNT = cnt
    return _COUNT


def _host_stitch(outs):
    count = _fold_count()
    res = np.empty((4, 1, 128, 128), np.float32)
    for n in range(4):
        acc = np.zeros((128, 128), np.float32)
        acc[0:68, :] += outs[2 * n]
        acc[61:128, :] += outs[2 * n + 1][0:67][::-1]
        res[n, 0] = acc / count
    return res


def kernel(**inputs) -> np.ndarray:
    global LAST_RESULTS, LAST_EXEC_WALL_S
    st_op = _register_st_op()
    if "nc" not in _CACHE:
        _CACHE["nc"] = _build_nc(st_op)
    nc = _CACHE["nc"]
    in_maps = _host_inputs(inputs)
    t0 = time.time()
    res = run_bass_kernel_spmd(nc, in_maps, core_ids=list(range(NCORES)))
    LAST_EXEC_WALL_S = time.time() - t0
    LAST_RESULTS = res
    outs = [res.results[c]["out"] for c in range(NCORES)]
    return _host_stitch(outs)
